# revision 1
# baseline (speedup 1.0000x reference)
"""HSTU block kernel for 8 TRN2 NeuronCores (Bass/Tile, fp32r matmuls).

Sharding: phase 1 (f1 + attention + u-gating) is data-parallel over batch
(B=2) x tensor-parallel over head groups (4 heads/core). Phase 2
(ln1 -> f2 -> +residual -> ln2) is row-parallel (512 rows/core). Host
gathers/reshards between the two launches; everything on-device is
feature-major so no transposes are ever needed.
"""
import os
import numpy as np

import concourse.bacc as bacc
import concourse.mybir as mybir
from concourse.tile import TileContext
from concourse.bass_utils import run_bass_kernel_spmd

fp32 = mybir.dt.float32
fp32r = mybir.dt.float32r
AF = mybir.ActivationFunctionType
ALU = mybir.AluOpType

B, S, D, H, M = 2, 2048, 1024, 16, 4096
HD = D // H          # 64
EPS = 1e-5
P = 128
NB = S // P          # 16 seq blocks of 128
NG = S // 512        # 4 q-groups of 512
DC = 4 * HD          # 256 features per core in phase 1

_CACHE = {}


# ---------------------------------------------------------------- kernel A
def build_kernel_a(plan, jmin, n_strip, n_partial):
    """plan[g] = [(kt, lead, trail, cls4)], strip holds tiles jmin..jmin+n_strip-1."""
    nc = bacc.Bacc("TRN2", target_bir_lowering=False, debug=False, num_devices=8)

    xT = nc.dram_tensor("xT", [D, S], fp32, kind="ExternalInput")
    w1T_qku = nc.dram_tensor("w1T_qku", [D, 768], fp32, kind="ExternalInput")
    w1T_v = nc.dram_tensor("w1T_v", [D, DC], fp32, kind="ExternalInput")
    b1_2d = nc.dram_tensor("b1_2d", [P, 6], fp32, kind="ExternalInput")
    b1v_bc = nc.dram_tensor("b1v_bc", [P, DC], fp32, kind="ExternalInput")
    inv128 = nc.dram_tensor("inv128", [P, P], fp32, kind="ExternalInput")
    ident = nc.dram_tensor("ident", [P, P], fp32, kind="ExternalInput")
    strip = nc.dram_tensor("strip", [P, n_strip * P], fp32, kind="ExternalInput")
    zeros_t = nc.dram_tensor("zeros_t", [P, 512], fp32, kind="ExternalInput")
    parts = nc.dram_tensor("parts", [max(n_partial, 1), P, P], fp32,
                           kind="ExternalInput")
    yT_out = nc.dram_tensor("yT_out", [DC, S], fp32, kind="ExternalOutput")

    with TileContext(nc) as tc:
        with tc.tile_pool(name="const", bufs=1) as cpool, \
             tc.tile_pool(name="wpool", bufs=2) as wpool, \
             tc.tile_pool(name="big", bufs=1) as big, \
             tc.tile_pool(name="att", bufs=4) as apool, \
             tc.tile_pool(name="out", bufs=3) as opool, \
             tc.tile_pool(name="ps", bufs=1, space="PSUM") as ps:

            # critical-path first: slab-0 x columns, then fc-major weight cols
            xs0 = []
            for k in range(8):
                t = wpool.tile([P, 512], fp32r, name="xs", tag=f"xs{k}", bufs=2)
                nc.sync.dma_start(t[:], xT[k * P:(k + 1) * P, 0:512].bitcast(fp32r))
                xs0.append(t)
            wq = []
            for k in range(8):
                t = big.tile([P, 768], fp32r, name=f"wq{k}", tag=f"wq{k}")
                nc.sync.dma_start(t[:],
                                  w1T_qku[k * P:(k + 1) * P, :].bitcast(fp32r))
                wq.append(t)
            strip_sb = big.tile([P, n_strip * P], fp32r, name="strip_sb", tag="strip")
            nc.sync.dma_start(strip_sb[:], strip[:].bitcast(fp32r))
            ident_sb = cpool.tile([P, P], fp32r, name="ident_sb")
            nc.sync.dma_start(ident_sb[:], ident[:].bitcast(fp32r))
            inv128_sb = cpool.tile([P, P], fp32r, name="inv128_sb")
            nc.sync.dma_start(inv128_sb[:], inv128[:].bitcast(fp32r))
            b1v_sb = cpool.tile([P, DC], fp32r, name="b1v_sb")
            nc.sync.dma_start(b1v_sb[:], b1v_bc[:].bitcast(fp32r))
            b1_sb = cpool.tile([P, 6], fp32, name="b1_sb")
            nc.sync.dma_start(b1_sb[:], b1_2d[:])
            zeros_sb = cpool.tile([P, 512], fp32r, name="zeros_sb")
            nc.sync.dma_start(zeros_sb[:], zeros_t[:].bitcast(fp32r))
            parts_sb = []
            for i in range(n_partial):
                t = cpool.tile([P, P], fp32r, name=f"part{i}", tag=f"part{i}")
                nc.sync.dma_start(t[:], parts[i].bitcast(fp32r))
                parts_sb.append(t)
            # persistent v weights
            wv = []
            for k in range(8):
                t = big.tile([P, DC], fp32r, name=f"w1v{k}", tag=f"w1v{k}")
                nc.sync.dma_start(t[:], w1T_v[k * P:(k + 1) * P, :].bitcast(fp32r))
                wv.append(t)

            # ---------------- f1: qku (feature-major) + v (natural) --------
            # k/v persist (attention history); q/u/x rotate per slab
            kh = [big.tile([64, S], fp32r, name=f"khh{h}", tag=f"khh{h}")
                  for h in range(4)]
            v_sb = [big.tile([P, DC], fp32r, name=f"v{sc}", tag=f"v{sc}")
                    for sc in range(NB)]

            def f1_slab(sg):
                sl = slice(sg * 512, (sg + 1) * 512)
                if sg == 0:
                    xs = xs0
                else:
                    xs = []
                    for k in range(8):
                        t = wpool.tile([P, 512], fp32r, name="xs", tag=f"xs{k}",
                                       bufs=2)
                        nc.sync.dma_start(
                            t[:], xT[k * P:(k + 1) * P, sl].bitcast(fp32r))
                        xs.append(t)
                qs = [wpool.tile([64, 512], fp32r, name="qs", tag=f"qs{h}",
                                 bufs=2) for h in range(4)]
                us = [wpool.tile([P, 512], fp32, name="us", tag=f"us{p}",
                                 bufs=2) for p in range(2)]
                for fc in range(6):
                    pt = ps.tile([P, 512], fp32, name="f1ps", tag="mm", bufs=4)
                    for k in range(8):
                        nc.tensor.matmul(pt[:], wq[k][:, fc * P:(fc + 1) * P],
                                         xs[k][:],
                                         start=(k == 0), stop=(k == 7))
                    if fc >= 4:
                        nc.scalar.activation(us[fc - 4][:], pt[:],
                                             AF.Silu, bias=b1_sb[:, fc:fc + 1],
                                             scale=1.0)
                    elif fc < 2:
                        nc.scalar.activation(qs[2 * fc][:], pt[0:64, :],
                                             AF.Silu, bias=b1_sb[0:64, fc:fc + 1],
                                             scale=1.0)
                        nc.scalar.activation(qs[2 * fc + 1][:], pt[64:P, :],
                                             AF.Silu, bias=b1_sb[64:P, fc:fc + 1],
                                             scale=1.0)
                    else:
                        ha = kh[2 * (fc - 2)]
                        hb = kh[2 * (fc - 2) + 1]
                        nc.scalar.activation(ha[:, sl], pt[0:64, :],
                                             AF.Silu, bias=b1_sb[0:64, fc:fc + 1],
                                             scale=1.0)
                        nc.scalar.activation(hb[:, sl], pt[64:P, :],
                                             AF.Silu, bias=b1_sb[64:P, fc:fc + 1],
                                             scale=1.0)
                for sc in range(4 * sg, 4 * sg + 4):
                    pt = ps.tile([P, DC], fp32, name="f1vps", tag="mm", bufs=4)
                    for k in range(8):
                        nc.tensor.matmul(pt[:],
                                         xs[k][:, (sc - 4 * sg) * P:
                                               (sc - 4 * sg + 1) * P],
                                         wv[k][:],
                                         start=(k == 0), stop=False)
                    nc.tensor.matmul(pt[:], inv128_sb[:], b1v_sb[:],
                                     start=False, stop=True)
                    nc.scalar.activation(v_sb[sc][:], pt[:], AF.Silu, scale=1.0)
                return qs, us

            # ---------------- attention (per head) ----------------
            def attn_pair(p_idx, g, qs, us):
                uT = us[p_idx]
                if True:
                    kts = plan[g]
                    avps = []
                    for hp in range(2):  # head within pair
                        h = 2 * p_idx + hp
                        avp = ps.tile([64, 512], fp32, name="avps",
                                      tag=f"avps{hp}", bufs=2)
                        avps.append(avp)
                        first = True
                        for ki, (kt, lead, trail, cls4) in enumerate(kts):
                            off, end = lead * P, 512 - trail * P
                            pe_bias = (ki % 2 == 0)
                            spp = ps.tile([P, 512], fp32, name="sps",
                                          tag="mm", bufs=4)
                            sb0 = (4 * g - kt + 15 - jmin) * P
                            nc.tensor.matmul(
                                spp[:, off:end],
                                kh[h][:, kt * P:(kt + 1) * P],
                                qs[h][:, off:end],
                                start=True, stop=not pe_bias)
                            if pe_bias:
                                nc.tensor.matmul(spp[:, off:end], ident_sb[:],
                                                 strip_sb[:, sb0 + off:sb0 + end],
                                                 start=False, stop=True)
                            att = apool.tile([P, 512], fp32r, name="att",
                                             tag="att", bufs=6)
                            if first and off > 0:
                                nc.vector.tensor_copy(att[:, 0:off],
                                                      zeros_sb[:, 0:off])
                            if first and end < 512:
                                nc.vector.tensor_copy(att[:, end:512],
                                                      zeros_sb[:, end:512])
                            if pe_bias:
                                nc.scalar.activation(att[:, off:end],
                                                     spp[:, off:end],
                                                     AF.Silu, scale=1.0)
                            else:
                                stmp = apool.tile([P, 512], fp32, name="stmp",
                                                  tag="stmp", bufs=3)
                                nc.vector.tensor_tensor(
                                    stmp[:, off:end], spp[:, off:end],
                                    strip_sb[:, sb0 + off:sb0 + end].bitcast(fp32),
                                    ALU.add)
                                nc.scalar.activation(att[:, off:end],
                                                     stmp[:, off:end],
                                                     AF.Silu, scale=1.0)
                            for j in range(lead, 4 - trail):
                                c = cls4[j]
                                if c == 0:
                                    nc.vector.tensor_copy(
                                        att[:, j * P:(j + 1) * P],
                                        zeros_sb[:, 0:P])
                                elif c >= 2:
                                    nc.vector.tensor_tensor(
                                        att[:, j * P:(j + 1) * P],
                                        att[:, j * P:(j + 1) * P],
                                        parts_sb[c - 2][:], ALU.mult)
                            ao, ae = (0, 512) if first else (off, end)
                            nc.tensor.matmul(
                                avp[:, ao:ae],
                                v_sb[kt][:, h * 64:(h + 1) * 64],
                                att[:, ao:ae],
                                start=first, stop=(kt == kts[-1][0]))
                            first = False
                    yg = opool.tile([P, 512], fp32, name="yg", tag="yg")
                    gsl = slice(g * 512, (g + 1) * 512)
                    if kts:
                        nc.vector.tensor_tensor(yg[0:64, :], avps[0][:],
                                                uT[0:64, :], ALU.mult)
                        nc.vector.tensor_tensor(yg[64:P, :], avps[1][:],
                                                uT[64:P, :], ALU.mult)
                    else:
                        nc.vector.tensor_copy(yg[:], zeros_sb[:])
                    nc.sync.dma_start(
                        yT_out[p_idx * P:(p_idx + 1) * P, gsl], yg[:])

            for sg in range(NG):
                qs, us = f1_slab(sg)
                attn_pair(0, sg, qs, us)
                attn_pair(1, sg, qs, us)
    return nc


# ---------------------------------------------------------------- kernel B
def build_kernel_b():
    nc = bacc.Bacc("TRN2", target_bir_lowering=False, debug=False, num_devices=8)

    yT = nc.dram_tensor("yT", [D, 512], fp32, kind="ExternalInput")
    xTs = nc.dram_tensor("xTs", [D, 512], fp32, kind="ExternalInput")
    w2T = nc.dram_tensor("w2T", [D, D], fp32, kind="ExternalInput")
    gvec = nc.dram_tensor("gvec", [P, 32], fp32, kind="ExternalInput")
    # gvec cols: 0-7 g1, 8-15 beta1, 16-23 g2, 24-31 beta2 (per 128-chunk)
    b2c = nc.dram_tensor("b2c", [P, 8], fp32, kind="ExternalInput")
    ones_p = nc.dram_tensor("ones_p", [P, 1], fp32, kind="ExternalInput")
    ones_r = nc.dram_tensor("ones_r", [1, P], fp32, kind="ExternalInput")
    outT = nc.dram_tensor("outT", [D, 512], fp32, kind="ExternalOutput")

    with TileContext(nc) as tc:
        with tc.tile_pool(name="const", bufs=1) as cpool, \
             tc.tile_pool(name="big", bufs=1) as big, \
             tc.tile_pool(name="wpool", bufs=3) as wpool, \
             tc.tile_pool(name="tmp", bufs=3) as tp, \
             tc.tile_pool(name="ps", bufs=1, space="PSUM") as ps:

            onesp = cpool.tile([P, 1], fp32r, name="onesp")
            nc.sync.dma_start(onesp[:], ones_p[:].bitcast(fp32r))
            onesr = cpool.tile([1, P], fp32r, name="onesr")
            nc.sync.dma_start(onesr[:], ones_r[:].bitcast(fp32r))
            gv = cpool.tile([P, 32], fp32, name="gv")
            nc.sync.dma_start(gv[:], gvec[:])
            b2s = cpool.tile([P, 8], fp32, name="b2s")
            nc.sync.dma_start(b2s[:], b2c[:])

            yt = []
            xts = []
            for i in range(8):
                t = big.tile([P, 512], fp32r, name=f"yt{i}", tag=f"yt{i}")
                nc.sync.dma_start(t[:], yT[i * P:(i + 1) * P, :].bitcast(fp32r))
                yt.append(t)
                t2 = big.tile([P, 512], fp32, name=f"xts{i}", tag=f"xts{i}")
                nc.sync.dma_start(t2[:], xTs[i * P:(i + 1) * P, :])
                xts.append(t2)
            w2sb = []
            for k in range(8):
                t = big.tile([P, D], fp32r, name=f"w2sb{k}", tag=f"w2sb{k}")
                nc.sync.dma_start(t[:], w2T[k * P:(k + 1) * P, :].bitcast(fp32r))
                w2sb.append(t)

            def layernorm(src_r, gcol, becol, out_dt, tag):
                """src_r: 8 x [128,512] fp32r tiles; returns 8 out tiles."""
                psum_s = ps.tile([1, 512], fp32, name=f"ls{tag}", tag="ls")
                psum_q = ps.tile([1, 512], fp32, name=f"lq{tag}", tag="lq")
                for i in range(8):
                    nc.tensor.matmul(psum_s[:], onesp[:], src_r[i][:],
                                     start=(i == 0), stop=(i == 7))
                for i in range(8):
                    sq = tp.tile([P, 512], fp32r, name="sq", tag="sq")
                    nc.scalar.activation(sq[:], src_r[i][:].bitcast(fp32),
                                         AF.Square, scale=1.0)
                    nc.tensor.matmul(psum_q[:], onesp[:], sq[:],
                                     start=(i == 0), stop=(i == 7))
                mu = tp.tile([1, 512], fp32, name=f"mu{tag}", tag="vec")
                nc.vector.tensor_scalar(mu[:], psum_s[:], 1.0 / D, None, ALU.mult)
                msq = tp.tile([1, 512], fp32, name=f"msq{tag}", tag="vec")
                nc.vector.tensor_scalar(msq[:], psum_q[:], 1.0 / D, None, ALU.mult)
                var = tp.tile([1, 512], fp32, name=f"var{tag}", tag="vec")
                nc.vector.tensor_tensor(var[:], mu[:], mu[:], ALU.mult)
                nc.vector.tensor_tensor(var[:], msq[:], var[:], ALU.subtract)
                nc.vector.tensor_scalar(var[:], var[:], EPS, None, ALU.add)
                sd = tp.tile([1, 512], fp32, name=f"sd{tag}", tag="vec")
                nc.scalar.activation(sd[:], var[:], AF.Sqrt, scale=1.0)
                rstd = tp.tile([1, 512], fp32, name=f"rstd{tag}", tag="vec")
                nc.vector.reciprocal(rstd[:], sd[:])
                ar = tp.tile([1, 512], fp32r, name=f"ar{tag}", tag="vecr",
                             bufs=4)
                nc.vector.tensor_copy(ar[:], rstd[:])
                nb = tp.tile([1, 512], fp32, name=f"nb{tag}", tag="vec")
                nc.vector.tensor_tensor(nb[:], mu[:], rstd[:], ALU.mult)
                br = tp.tile([1, 512], fp32r, name=f"br{tag}", tag="vecr",
                             bufs=4)
                nc.vector.tensor_scalar(br[:], nb[:], -1.0, None, ALU.mult)
                pa = ps.tile([P, 512], fp32, name=f"pa{tag}", tag="pa")
                pb = ps.tile([P, 512], fp32, name=f"pb{tag}", tag="pb")
                nc.tensor.matmul(pa[:], onesr[:], ar[:], start=True, stop=True)
                nc.tensor.matmul(pb[:], onesr[:], br[:], start=True, stop=True)
                outs = []
                for i in range(8):
                    t = tp.tile([P, 512], fp32, name="lnt", tag="lnt")
                    nc.vector.tensor_tensor(t[:], src_r[i][:].bitcast(fp32),
                                            pa[:], ALU.mult)
                    nc.vector.tensor_tensor(t[:], t[:], pb[:], ALU.add)
                    o = tp.tile([P, 512], out_dt, name=f"lno{tag}",
                                tag=f"lno{tag}{i}", bufs=1)
                    nc.vector.tensor_scalar(o[:], t[:], gv[:, gcol + i:gcol + i + 1],
                                            gv[:, becol + i:becol + i + 1],
                                            ALU.mult, ALU.add)
                    outs.append(o)
                return outs

            yln = layernorm(yt, 0, 8, fp32r, "a")

            # f2 + residual (+ b2); k-outer so it starts when yln[0] lands
            t2r = [None] * 8
            for half in range(2):
                fcs = [4 * half + j for j in range(4)]
                pts = {fc: ps.tile([P, 512], fp32, name=f"f2ps{fc}",
                                   tag=f"f2ps{fc % 4}", bufs=1) for fc in fcs}
                for k in range(8):
                    for fc in fcs:
                        nc.tensor.matmul(pts[fc][:],
                                         w2sb[k][:, fc * P:(fc + 1) * P],
                                         yln[k][:],
                                         start=(k == 0), stop=(k == 7))
                for fc in fcs:
                    tb = tp.tile([P, 512], fp32, name="tb", tag="tb")
                    nc.vector.tensor_scalar(tb[:], pts[fc][:],
                                            b2s[:, fc:fc + 1], None, ALU.add)
                    t = big.tile([P, 512], fp32r, name=f"t2r{fc}",
                                 tag=f"t2r{fc}")
                    nc.vector.tensor_tensor(t[:], tb[:], xts[fc][:], ALU.add)
                    t2r[fc] = t

            out_f = layernorm(t2r, 16, 24, fp32, "b")
            for i in range(8):
                nc.sync.dma_start(outT[i * P:(i + 1) * P, :], out_f[i][:])
    return nc


# ---------------------------------------------------------------- host side
def _classify_mask(mask):
    keep = (mask.reshape(S, S) >= 0)
    block_cls = [[0] * NB for _ in range(NB)]  # [kt][qb]
    partials = []
    pmap = {}
    for kt in range(NB):
        for qb in range(NB):
            sub = keep[qb * P:(qb + 1) * P, kt * P:(kt + 1) * P]
            if sub.all():
                block_cls[kt][qb] = 1
            elif not sub.any():
                block_cls[kt][qb] = 0
            else:
                tile = np.ascontiguousarray(sub.T.astype(np.float32))
                key = tile.tobytes()
                if key not in pmap:
                    pmap[key] = len(partials)
                    partials.append(tile)
                block_cls[kt][qb] = 2 + pmap[key]
    return block_cls, partials



def _plan_attn(block_cls):
    """Per (g, kt): lead/trail skip blocks + strip tile range used."""
    plan = {}
    jmin, jmax = 31, 0
    for g in range(NG):
        kts = []
        for kt in range(NB):
            cls4 = [block_cls[kt][4 * g + j] for j in range(4)]
            if all(c == 0 for c in cls4):
                continue
            lead = 0
            while cls4[lead] == 0:
                lead += 1
            trail = 0
            while cls4[3 - trail] == 0:
                trail += 1
            jj0 = 4 * g - kt + 15
            jmin = min(jmin, jj0 + lead)
            jmax = max(jmax, jj0 + 3 - trail)
            kts.append((kt, lead, trail, cls4))
        plan[g] = kts
    if jmin > jmax:
        jmin, jmax = 0, 0
    return plan, jmin, jmax

def _get_compiled(mask_bytes, mask):
    if mask_bytes in _CACHE:
        return _CACHE[mask_bytes]
    block_cls, partials = _classify_mask(mask)
    plan, jmin, jmax = _plan_attn(block_cls)
    n_strip = jmax - jmin + 1
    nca = build_kernel_a(plan, jmin, n_strip, len(partials))
    nca.compile()
    ncb = build_kernel_b()
    ncb.compile()
    _CACHE[mask_bytes] = (nca, ncb, jmin, n_strip, partials)
    return _CACHE[mask_bytes]


def kernel(x, mask, w1, b1, w2, b2, g1, beta1, g2, beta2, pos_w):
    x = np.asarray(x, np.float32)
    w1 = np.asarray(w1, np.float32)
    b1 = np.asarray(b1, np.float32)
    w2 = np.asarray(w2, np.float32)
    b2 = np.asarray(b2, np.float32)
    g1 = np.asarray(g1, np.float32)
    beta1 = np.asarray(beta1, np.float32)
    g2 = np.asarray(g2, np.float32)
    beta2 = np.asarray(beta2, np.float32)
    pos_w = np.asarray(pos_w, np.float32)
    mask_np = np.asarray(mask)

    nca, ncb, jmin, n_strip, partials = _get_compiled(mask_np.tobytes(), mask_np)

    trace = bool(int(os.environ.get("HSTU_TRACE", "0")))
    strip = np.zeros((P, n_strip * P), np.float32)
    pidx = np.arange(P)[:, None]
    fidx = np.arange(P)[None, :]
    for i in range(n_strip):
        jj = jmin + i
        base = M - 1 - P * (jj - 15)
        strip[:, i * P:(i + 1) * P] = pos_w[base + pidx - fidx]
    ident = np.eye(P, dtype=np.float32)
    inv128 = np.full((P, P), 1.0 / P, np.float32)
    zeros_t = np.zeros((P, 512), np.float32)
    parts_arr = (np.stack(partials) if partials
                 else np.zeros((1, P, P), np.float32))

    xT = [np.ascontiguousarray(x[b].T) for b in range(B)]
    in_maps_a = []
    for c in range(8):
        b, hg = divmod(c, 4)
        heads = [4 * hg + i for i in range(4)]
        rows_q = np.concatenate([np.arange(D + h * HD, D + (h + 1) * HD)
                                 for h in heads])
        rows_k = np.concatenate([np.arange(2 * D + h * HD, 2 * D + (h + 1) * HD)
                                 for h in heads])
        rows_u = np.concatenate([np.arange(h * HD, (h + 1) * HD) for h in heads])
        rows_v = np.concatenate([np.arange(3 * D + h * HD, 3 * D + (h + 1) * HD)
                                 for h in heads])
        rows_qku = np.concatenate([rows_q, rows_k, rows_u])
        b1_2d = np.ascontiguousarray(
            b1[rows_qku].reshape(6, P).T)
        in_maps_a.append(dict(
            xT=xT[b],
            w1T_qku=np.ascontiguousarray(w1[rows_qku].T),
            w1T_v=np.ascontiguousarray(w1[rows_v].T),
            b1_2d=b1_2d,
            b1v_bc=np.ascontiguousarray(
                np.broadcast_to(b1[rows_v][None, :], (P, DC))),
            inv128=inv128, ident=ident, strip=strip, zeros_t=zeros_t,
            parts=parts_arr,
        ))
    res_a = run_bass_kernel_spmd(nca, in_maps_a, core_ids=list(range(8)),
                                 trace=trace)

    yT_full = [np.empty((D, S), np.float32) for _ in range(B)]
    for c in range(8):
        b, hg = divmod(c, 4)
        yT_full[b][hg * DC:(hg + 1) * DC] = res_a.results[c]["yT_out"]

    w2T = np.ascontiguousarray(w2.T)
    b2c = np.ascontiguousarray(b2.reshape(8, P).T)
    gvec = np.concatenate([g1.reshape(8, P).T, beta1.reshape(8, P).T,
                           g2.reshape(8, P).T, beta2.reshape(8, P).T], axis=1)
    gvec = np.ascontiguousarray(gvec)
    ones_p = np.ones((P, 1), np.float32)
    ones_r = np.ones((1, P), np.float32)
    in_maps_b = []
    for c in range(8):
        b, qc = divmod(c, 4)
        sl = slice(qc * 512, (qc + 1) * 512)
        in_maps_b.append(dict(
            yT=np.ascontiguousarray(yT_full[b][:, sl]),
            xTs=np.ascontiguousarray(xT[b][:, sl]),
            w2T=w2T, b2c=b2c, gvec=gvec,
            ones_p=ones_p, ones_r=ones_r,
        ))
    res_b = run_bass_kernel_spmd(ncb, in_maps_b, core_ids=list(range(8)),
                                 trace=trace)

    out = np.empty((B, S, D), np.float32)
    for c in range(8):
        b, qc = divmod(c, 4)
        out[b, qc * 512:(qc + 1) * 512] = res_b.results[c]["outT"].T
    kernel.last_results = (res_a, res_b)
    return out



# revision 14
# speedup vs baseline: 1.2038x; 1.2038x over previous
"""HSTU block kernel for 8 TRN2 NeuronCores (Bass/Tile, bf16 matmuls).

Sharding: phase 1 (f1 + attention + u-gating) is data-parallel over batch
(B=2) x tensor-parallel over head groups (4 heads/core). Phase 2
(ln1 -> f2 -> +residual -> ln2) is row-parallel (512 rows/core). Host
gathers/reshards between the two launches; everything on-device is
feature-major so no transposes are ever needed.

Phase 2 exploits linearity: f2 runs on raw (un-normalized) y, with the
-mu correction folded in as a rank-1 matmul into the same PSUM
accumulation, so the GEMM overlaps the layernorm stats instead of
waiting on them.
"""
import os
import numpy as np
import ml_dtypes

import concourse.bacc as bacc
import concourse.mybir as mybir
from concourse.tile import TileContext
from concourse.bass_utils import run_bass_kernel_spmd

fp32 = mybir.dt.float32
fp32r = mybir.dt.float32r
bf16 = mybir.dt.bfloat16
AF = mybir.ActivationFunctionType
ALU = mybir.AluOpType
nbf = ml_dtypes.bfloat16

B, S, D, H, M = 2, 2048, 1024, 16, 4096
HD = D // H          # 64
EPS = 1e-5
P = 128
NB = S // P          # 16 seq blocks of 128
NG = S // 512        # 4 q-groups of 512
DC = 4 * HD          # 256 features per core in phase 1

_CACHE = {}


# ---------------------------------------------------------------- kernel A
def build_kernel_a(plan, jmin, n_strip, n_partial):
    """plan[g] = [(kt, lead, trail, cls4)], strip holds tiles jmin..jmin+n_strip-1."""
    nc = bacc.Bacc("TRN2", target_bir_lowering=False, debug=False, num_devices=8)

    xTr = nc.dram_tensor("xTr", [P, NG, 8, 512], bf16, kind="ExternalInput")
    w1qr = nc.dram_tensor("w1qr", [P, 8, 768], bf16, kind="ExternalInput")
    wvr = nc.dram_tensor("wvr", [P, 8, DC], bf16, kind="ExternalInput")
    b1_2d = nc.dram_tensor("b1_2d", [P, 6], fp32, kind="ExternalInput")
    b1v_bc = nc.dram_tensor("b1v_bc", [P, DC], bf16, kind="ExternalInput")
    inv128 = nc.dram_tensor("inv128", [P, P], bf16, kind="ExternalInput")
    ident = nc.dram_tensor("ident", [P, P], bf16, kind="ExternalInput")
    strip = nc.dram_tensor("strip", [P, n_strip * P], bf16, kind="ExternalInput")
    zeros_t = nc.dram_tensor("zeros_t", [P, 512], bf16, kind="ExternalInput")
    parts = nc.dram_tensor("parts", [P, max(n_partial, 1) * P], bf16,
                           kind="ExternalInput")
    yT_out = nc.dram_tensor("yT_out", [DC, S], bf16, kind="ExternalOutput")

    with TileContext(nc) as tc:
        with tc.tile_pool(name="const", bufs=1) as cpool, \
             tc.tile_pool(name="wpool", bufs=2) as wpool, \
             tc.tile_pool(name="big", bufs=1) as big, \
             tc.tile_pool(name="att", bufs=4) as apool, \
             tc.tile_pool(name="out", bufs=3) as opool, \
             tc.tile_pool(name="ps", bufs=1, space="PSUM") as ps:

            # few, large DMAs (one dma_start fans out over all 16 SDMA
            # engines; <=64KB transfers are descriptor-dominated)
            xs0 = wpool.tile([P, 8 * 512], bf16, name="xs", tag="xs", bufs=2)
            nc.sync.dma_start(xs0[:], xTr[:, 0])
            wq_all = big.tile([P, 8 * 768], bf16, name="wq_all", tag="wq")
            nc.sync.dma_start(wq_all[:], w1qr[:])
            b1_sb = cpool.tile([P, 6], fp32, name="b1_sb")
            nc.sync.dma_start(b1_sb[:], b1_2d[:])
            wv_all = big.tile([P, 8 * DC], bf16, name="wv_all", tag="wv")
            nc.sync.dma_start(wv_all[:], wvr[:])
            b1v_sb = cpool.tile([P, DC], bf16, name="b1v_sb")
            nc.sync.dma_start(b1v_sb[:], b1v_bc[:])
            inv128_sb = cpool.tile([P, P], bf16, name="inv128_sb")
            nc.sync.dma_start(inv128_sb[:], inv128[:])
            strip_sb = big.tile([P, n_strip * P], bf16, name="strip_sb",
                                tag="strip")
            nc.sync.dma_start(strip_sb[:], strip[:])
            ident_sb = cpool.tile([P, P], bf16, name="ident_sb")
            nc.sync.dma_start(ident_sb[:], ident[:])
            parts_all = cpool.tile([P, max(n_partial, 1) * P], bf16,
                                   name="parts_all")
            nc.sync.dma_start(parts_all[:], parts[:])
            zeros_sb = cpool.tile([P, 512], bf16, name="zeros_sb")
            nc.sync.dma_start(zeros_sb[:], zeros_t[:])

            # ---------------- f1: qku (feature-major) + v (natural) --------
            # k/v persist (attention history); q/u/x rotate per slab.
            # kh[p] stacks the head pair: rows 0:64 = head 2p, 64:128 = 2p+1.
            kh = [big.tile([P, S], bf16, name=f"khh{p}", tag=f"khh{p}")
                  for p in range(2)]
            v_sb = [big.tile([P, DC], bf16, name=f"v{sc}", tag=f"v{sc}")
                    for sc in range(NB)]

            def f1_slab(sg):
                if sg == 0:
                    xs = xs0
                else:
                    xs = wpool.tile([P, 8 * 512], bf16, name="xs", tag="xs",
                                    bufs=2)
                    nc.sync.dma_start(xs[:], xTr[:, sg])
                sl = slice(sg * 512, (sg + 1) * 512)
                # qs[p]: head pair stacked like kh
                qs = [wpool.tile([P, 512], bf16, name="qs", tag=f"qs{p}",
                                 bufs=2) for p in range(2)]
                us = [wpool.tile([P, 512], bf16, name="us", tag=f"us{p}",
                                 bufs=2) for p in range(2)]
                for fc in range(6):
                    pt = ps.tile([P, 512], fp32, name="f1ps", tag="f1mm",
                                 bufs=2)
                    for k in range(8):
                        nc.tensor.matmul(
                            pt[:],
                            wq_all[:, k * 768 + fc * P:k * 768 + (fc + 1) * P],
                            xs[:, k * 512:(k + 1) * 512],
                            start=(k == 0), stop=(k == 7))
                    if fc >= 4:
                        nc.scalar.activation(us[fc - 4][:], pt[:],
                                             AF.Silu, bias=b1_sb[:, fc:fc + 1],
                                             scale=1.0)
                    elif fc < 2:
                        nc.scalar.activation(qs[fc][:], pt[:],
                                             AF.Silu, bias=b1_sb[:, fc:fc + 1],
                                             scale=1.0)
                    else:
                        nc.scalar.activation(kh[fc - 2][:, sl], pt[:],
                                             AF.Silu, bias=b1_sb[:, fc:fc + 1],
                                             scale=1.0)
                for sc in range(4 * sg, 4 * sg + 4):
                    pt = ps.tile([P, DC], fp32, name="f1vps", tag="f1mm",
                                 bufs=2)
                    j = sc - 4 * sg
                    for k in range(8):
                        nc.tensor.matmul(
                            pt[:],
                            xs[:, k * 512 + j * P:k * 512 + (j + 1) * P],
                            wv_all[:, k * DC:(k + 1) * DC],
                            start=(k == 0), stop=False)
                    nc.tensor.matmul(pt[:], inv128_sb[:], b1v_sb[:],
                                     start=False, stop=True)
                    nc.scalar.activation(v_sb[sc][:], pt[:], AF.Silu, scale=1.0)
                return qs, us

            # ---------------- attention (per head pair) ----------------
            def attn_pair(p_idx, g, qs, us):
                kts = plan[g]
                gsl = slice(g * 512, (g + 1) * 512)
                avps = [ps.tile([64, 512], fp32, name="avp", tag=f"avph{hp}",
                                bufs=1) for hp in range(2)]
                first = True
                for ki, (kt, lead, trail, cls4) in enumerate(kts):
                    off, end = lead * P, 512 - trail * P
                    pe_bias = (ki % 4 == 0)
                    spp = ps.tile([P, 1024], fp32, name="sps", tag="spmm",
                                  bufs=2)
                    sb0 = (4 * g - kt + 15 - jmin) * P
                    for hp in range(2):
                        o2 = hp * 512
                        nc.tensor.matmul(
                            spp[:, o2 + off:o2 + end],
                            kh[p_idx][hp * 64:(hp + 1) * 64, kt * P:(kt + 1) * P],
                            qs[p_idx][hp * 64:(hp + 1) * 64, off:end],
                            start=True, stop=not pe_bias)
                        if pe_bias:
                            nc.tensor.matmul(spp[:, o2 + off:o2 + end],
                                             ident_sb[:],
                                             strip_sb[:, sb0 + off:sb0 + end],
                                             start=False, stop=True)
                    att = apool.tile([P, 1024], bf16, name="att",
                                     tag="att", bufs=6)
                    if first and off > 0:
                        for hp in range(2):
                            nc.vector.tensor_copy(att[:, hp * 512:hp * 512 + off],
                                                  zeros_sb[:, 0:off])
                    if first and end < 512:
                        for hp in range(2):
                            nc.vector.tensor_copy(
                                att[:, hp * 512 + end:(hp + 1) * 512],
                                zeros_sb[:, end:512])
                    if pe_bias:
                        if off == 0 and end == 512:
                            nc.scalar.activation(att[:], spp[:],
                                                 AF.Silu, scale=1.0)
                        else:
                            for hp in range(2):
                                o2 = hp * 512
                                nc.scalar.activation(att[:, o2 + off:o2 + end],
                                                     spp[:, o2 + off:o2 + end],
                                                     AF.Silu, scale=1.0)
                    else:
                        stmp = apool.tile([P, 1024], bf16, name="stmp",
                                          tag="stmp", bufs=3)
                        for hp in range(2):
                            o2 = hp * 512
                            nc.vector.tensor_tensor(
                                stmp[:, o2 + off:o2 + end],
                                spp[:, o2 + off:o2 + end],
                                strip_sb[:, sb0 + off:sb0 + end],
                                ALU.add)
                        if off == 0 and end == 512:
                            nc.scalar.activation(att[:], stmp[:],
                                                 AF.Silu, scale=1.0)
                        else:
                            for hp in range(2):
                                o2 = hp * 512
                                nc.scalar.activation(att[:, o2 + off:o2 + end],
                                                     stmp[:, o2 + off:o2 + end],
                                                     AF.Silu, scale=1.0)
                    for hp in range(2):
                        o2 = hp * 512
                        for j in range(lead, 4 - trail):
                            c = cls4[j]
                            if c == 0:
                                nc.vector.tensor_copy(
                                    att[:, o2 + j * P:o2 + (j + 1) * P],
                                    zeros_sb[:, 0:P])
                            elif c >= 2:
                                nc.vector.tensor_tensor(
                                    att[:, o2 + j * P:o2 + (j + 1) * P],
                                    att[:, o2 + j * P:o2 + (j + 1) * P],
                                    parts_all[:, (c - 2) * P:(c - 1) * P],
                                    ALU.mult)
                    ao, ae = (0, 512) if first else (off, end)
                    for hp in range(2):
                        h = 2 * p_idx + hp
                        nc.tensor.matmul(
                            avps[hp][:, ao:ae],
                            v_sb[kt][:, h * 64:(h + 1) * 64],
                            att[:, hp * 512 + ao:hp * 512 + ae],
                            start=first, stop=(kt == kts[-1][0]))
                    first = False
                yg = opool.tile([P, 512], bf16, name="yg", tag="yg")
                if kts:
                    for hp in range(2):
                        nc.vector.tensor_tensor(
                            yg[hp * 64:(hp + 1) * 64, :], avps[hp][:],
                            us[p_idx][hp * 64:(hp + 1) * 64, :], ALU.mult)
                else:
                    nc.vector.tensor_copy(yg[:], zeros_sb[:])
                nc.gpsimd.dma_start(yT_out[p_idx * P:(p_idx + 1) * P, gsl],
                                    yg[:])

            for sg in range(NG):
                qs, us = f1_slab(sg)
                attn_pair(0, sg, qs, us)
                attn_pair(1, sg, qs, us)
    return nc


# ---------------------------------------------------------------- kernel B
def build_kernel_b():
    nc = bacc.Bacc("TRN2", target_bir_lowering=False, debug=False, num_devices=8)

    yr = nc.dram_tensor("yr", [P, 8, 512], bf16, kind="ExternalInput")
    w2r = nc.dram_tensor("w2r", [P, 8, D], bf16, kind="ExternalInput")
    xbr = nc.dram_tensor("xbr", [P, 8, 512], fp32, kind="ExternalInput")
    wsr = nc.dram_tensor("wsr", [1, D], fp32, kind="ExternalInput")
    gv2 = nc.dram_tensor("gv2", [P, 16], fp32, kind="ExternalInput")
    # gv2 cols: 0-7 g2, 8-15 beta2 (per 128-chunk)
    ones_b = nc.dram_tensor("ones_b", [P, P], bf16, kind="ExternalInput")
    ones_r = nc.dram_tensor("ones_r", [P, P], fp32, kind="ExternalInput")
    epsc = nc.dram_tensor("epsc", [P, 1], fp32, kind="ExternalInput")
    outT = nc.dram_tensor("outT", [D, 512], fp32, kind="ExternalOutput")

    with TileContext(nc) as tc:
        with tc.tile_pool(name="const", bufs=1) as cpool, \
             tc.tile_pool(name="big", bufs=1) as big, \
             tc.tile_pool(name="tmp", bufs=3) as tp, \
             tc.tile_pool(name="ps", bufs=1, space="PSUM") as ps:

            ytall = big.tile([P, 8 * 512], bf16, name="ytall", tag="yt")
            nc.sync.dma_start(ytall[:], yr[:])
            onesb = cpool.tile([P, P], bf16, name="onesb")
            nc.sync.dma_start(onesb[:], ones_b[:])
            w2all = big.tile([P, 8 * D], bf16, name="w2all", tag="w2")
            nc.sync.dma_start(w2all[:], w2r[:])
            xball = big.tile([P, 8 * 512], fp32, name="xball", tag="xb")
            nc.sync.dma_start(xball[:], xbr[:])
            ws_sb = cpool.tile([1, D], fp32r, name="ws_sb")
            nc.sync.dma_start(ws_sb[:], wsr[:].bitcast(fp32r))
            onesr = cpool.tile([P, P], fp32r, name="onesr")
            nc.sync.dma_start(onesr[:], ones_r[:].bitcast(fp32r))
            gv = cpool.tile([P, 16], fp32, name="gv")
            nc.sync.dma_start(gv[:], gv2[:])
            eps_sb = cpool.tile([P, 1], fp32, name="eps_sb")
            nc.sync.dma_start(eps_sb[:], epsc[:])

            def yt(i):
                return ytall[:, i * 512:(i + 1) * 512]

            # ---- ln-a stats, broadcast across partitions via ones matmul
            psum_s = ps.tile([P, 512], fp32, name="lsa", tag="lsa")
            psum_q = ps.tile([P, 512], fp32, name="lqa", tag="lqa")
            for i in range(8):
                nc.tensor.matmul(psum_s[:], onesb[:], yt(i),
                                 start=(i == 0), stop=(i == 7))
            for i in range(8):
                sq = tp.tile([P, 512], bf16, name="sq", tag="sq")
                nc.scalar.activation(sq[:], yt(i), AF.Square, scale=1.0)
                nc.tensor.matmul(psum_q[:], onesb[:], sq[:],
                                 start=(i == 0), stop=(i == 7))
            negmu = tp.tile([1, 512], fp32r, name="negmu", tag="vr", bufs=2)
            nc.vector.tensor_scalar(negmu[:], psum_s[0:1, :], -1.0 / D, None,
                                    ALU.mult)
            mu_b = tp.tile([P, 512], fp32, name="mu_b", tag="mub", bufs=2)
            nc.vector.tensor_scalar(mu_b[:], psum_s[:], 1.0 / D, None, ALU.mult)
            t1 = tp.tile([P, 512], fp32, name="t1", tag="t1", bufs=2)
            nc.vector.tensor_tensor(t1[:], mu_b[:], mu_b[:], ALU.mult)
            msq = tp.tile([P, 512], fp32, name="msq", tag="msq", bufs=2)
            nc.vector.tensor_scalar(msq[:], psum_q[:], 1.0 / D, None, ALU.mult)
            var = tp.tile([P, 512], fp32, name="var", tag="var", bufs=2)
            nc.vector.tensor_tensor(var[:], msq[:], t1[:], ALU.subtract)
            sd_b = tp.tile([P, 512], fp32, name="sd_b", tag="sda", bufs=2)
            nc.scalar.activation(sd_b[:], var[:], AF.Sqrt,
                                 bias=eps_sb[:, 0:1], scale=1.0)
            rstd_b = tp.tile([P, 512], fp32, name="rstd_b", tag="rsa", bufs=2)
            nc.vector.reciprocal_approx_fast(rstd_b[:], sd_b[:])

            # ---- f2 on raw y; -mu correction folded in as rank-1 matmul
            t2r = [None] * 8
            for half in range(2):
                fcs = [4 * half + j for j in range(4)]
                pts = {fc: ps.tile([P, 512], fp32, name=f"f2ps{fc}",
                                   tag=f"f2ps{fc % 4}", bufs=1) for fc in fcs}
                for k in range(8):
                    for fc in fcs:
                        nc.tensor.matmul(
                            pts[fc][:],
                            w2all[:, k * D + fc * P:k * D + (fc + 1) * P],
                            yt(k), start=(k == 0), stop=False)
                for fc in fcs:
                    nc.tensor.matmul(pts[fc][:],
                                     ws_sb[0:1, fc * P:(fc + 1) * P],
                                     negmu[:], start=False, stop=True)
                    t = big.tile([P, 512], fp32r, name=f"t2r{fc}",
                                 tag=f"t2r{fc}")
                    nc.vector.tensor_tensor(t[:], pts[fc][:], rstd_b[:],
                                            ALU.mult)
                    nc.vector.tensor_tensor(
                        t[:], t[:], xball[:, fc * 512:(fc + 1) * 512], ALU.add)
                    t2r[fc] = t

            # ---- ln-b stats (broadcast) + apply
            psum2_s = ps.tile([P, 512], fp32, name="lsb", tag="lsb")
            psum2_q = ps.tile([P, 512], fp32, name="lqb", tag="lqb")
            for i in range(8):
                nc.tensor.matmul(psum2_s[:], onesr[:], t2r[i][:],
                                 start=(i == 0), stop=(i == 7))
            for i in range(8):
                sq2 = tp.tile([P, 512], fp32r, name="sq2", tag="sq")
                nc.scalar.activation(sq2[:], t2r[i][:].bitcast(fp32),
                                     AF.Square, scale=1.0)
                nc.tensor.matmul(psum2_q[:], onesr[:], sq2[:],
                                 start=(i == 0), stop=(i == 7))
            mu2 = tp.tile([P, 512], fp32, name="mu2", tag="mub", bufs=2)
            nc.vector.tensor_scalar(mu2[:], psum2_s[:], 1.0 / D, None, ALU.mult)
            t12 = tp.tile([P, 512], fp32, name="t12", tag="t1", bufs=2)
            nc.vector.tensor_tensor(t12[:], mu2[:], mu2[:], ALU.mult)
            msq2 = tp.tile([P, 512], fp32, name="msq2", tag="msq", bufs=2)
            nc.vector.tensor_scalar(msq2[:], psum2_q[:], 1.0 / D, None, ALU.mult)
            var2 = tp.tile([P, 512], fp32, name="var2", tag="var", bufs=2)
            nc.vector.tensor_tensor(var2[:], msq2[:], t12[:], ALU.subtract)
            sd2 = tp.tile([P, 512], fp32, name="sd2", tag="sda", bufs=2)
            nc.scalar.activation(sd2[:], var2[:], AF.Sqrt,
                                 bias=eps_sb[:, 0:1], scale=1.0)
            rstd2 = tp.tile([P, 512], fp32, name="rstd2", tag="rsa", bufs=2)
            nc.vector.reciprocal_approx_fast(rstd2[:], sd2[:])
            nb2 = tp.tile([P, 512], fp32, name="nb2", tag="nb2")
            nc.vector.tensor_tensor(nb2[:], mu2[:], rstd2[:], ALU.mult)

            for i in range(8):
                t = tp.tile([P, 512], fp32, name="lnt", tag="lnt")
                nc.vector.tensor_tensor(t[:], t2r[i][:].bitcast(fp32),
                                        rstd2[:], ALU.mult)
                nc.vector.tensor_tensor(t[:], t[:], nb2[:], ALU.subtract)
                o = tp.tile([P, 512], fp32, name="lno", tag=f"lno{i}", bufs=1)
                nc.vector.tensor_scalar(o[:], t[:], gv[:, i:i + 1],
                                        gv[:, 8 + i:8 + i + 1],
                                        ALU.mult, ALU.add)
                nc.gpsimd.dma_start(outT[i * P:(i + 1) * P, :], o[:])
    return nc


# ---------------------------------------------------------------- host side
def _classify_mask(mask):
    keep = (mask.reshape(S, S) >= 0)
    block_cls = [[0] * NB for _ in range(NB)]  # [kt][qb]
    partials = []
    pmap = {}
    for kt in range(NB):
        for qb in range(NB):
            sub = keep[qb * P:(qb + 1) * P, kt * P:(kt + 1) * P]
            if sub.all():
                block_cls[kt][qb] = 1
            elif not sub.any():
                block_cls[kt][qb] = 0
            else:
                tile = np.ascontiguousarray(sub.T.astype(np.float32))
                key = tile.tobytes()
                if key not in pmap:
                    pmap[key] = len(partials)
                    partials.append(tile)
                block_cls[kt][qb] = 2 + pmap[key]
    return block_cls, partials


def _plan_attn(block_cls):
    """Per (g, kt): lead/trail skip blocks + strip tile range used."""
    plan = {}
    jmin, jmax = 31, 0
    for g in range(NG):
        kts = []
        for kt in range(NB):
            cls4 = [block_cls[kt][4 * g + j] for j in range(4)]
            if all(c == 0 for c in cls4):
                continue
            lead = 0
            while cls4[lead] == 0:
                lead += 1
            trail = 0
            while cls4[3 - trail] == 0:
                trail += 1
            jj0 = 4 * g - kt + 15
            jmin = min(jmin, jj0 + lead)
            jmax = max(jmax, jj0 + 3 - trail)
            kts.append((kt, lead, trail, cls4))
        plan[g] = kts
    if jmin > jmax:
        jmin, jmax = 0, 0
    return plan, jmin, jmax


def _get_compiled(mask_bytes, mask):
    if mask_bytes in _CACHE:
        return _CACHE[mask_bytes]
    block_cls, partials = _classify_mask(mask)
    plan, jmin, jmax = _plan_attn(block_cls)
    n_strip = jmax - jmin + 1
    nca = build_kernel_a(plan, jmin, n_strip, len(partials))
    nca.compile()
    ncb = build_kernel_b()
    ncb.compile()
    _CACHE[mask_bytes] = (nca, ncb, jmin, n_strip, partials)
    return _CACHE[mask_bytes]


def _chunk_major(a, nk):
    """[nk*128, F] -> [128, nk, F] (partition-major chunk layout)."""
    f = a.shape[1]
    return np.ascontiguousarray(a.reshape(nk, P, f).transpose(1, 0, 2))


def kernel(x, mask, w1, b1, w2, b2, g1, beta1, g2, beta2, pos_w):
    x = np.asarray(x, np.float32)
    w1 = np.asarray(w1, np.float32)
    b1 = np.asarray(b1, np.float32)
    w2 = np.asarray(w2, np.float32)
    b2 = np.asarray(b2, np.float32)
    g1 = np.asarray(g1, np.float32)
    beta1 = np.asarray(beta1, np.float32)
    g2 = np.asarray(g2, np.float32)
    beta2 = np.asarray(beta2, np.float32)
    pos_w = np.asarray(pos_w, np.float32)
    mask_np = np.asarray(mask)

    nca, ncb, jmin, n_strip, partials = _get_compiled(mask_np.tobytes(), mask_np)

    trace = bool(int(os.environ.get("HSTU_TRACE", "0")))
    strip = np.zeros((P, n_strip * P), np.float32)
    pidx = np.arange(P)[:, None]
    fidx = np.arange(P)[None, :]
    for i in range(n_strip):
        jj = jmin + i
        base = M - 1 - P * (jj - 15)
        strip[:, i * P:(i + 1) * P] = pos_w[base + pidx - fidx]
    ident = np.eye(P, dtype=nbf)
    inv128 = np.full((P, P), 1.0 / P, nbf)
    zeros_t = np.zeros((P, 512), nbf)
    parts_arr = (np.concatenate(partials, axis=1).astype(nbf) if partials
                 else np.zeros((P, P), nbf))

    xT = [np.ascontiguousarray(x[b].T) for b in range(B)]
    # [128, NG, 8, 512]: slab-major then k-chunk so one DMA covers a slab
    xTrs = [np.ascontiguousarray(
        t.astype(nbf).reshape(8, P, NG, 512).transpose(1, 2, 0, 3))
        for t in xT]
    in_maps_a = []
    for c in range(8):
        b, hg = divmod(c, 4)
        heads = [4 * hg + i for i in range(4)]
        rows_q = np.concatenate([np.arange(D + h * HD, D + (h + 1) * HD)
                                 for h in heads])
        rows_k = np.concatenate([np.arange(2 * D + h * HD, 2 * D + (h + 1) * HD)
                                 for h in heads])
        rows_u = np.concatenate([np.arange(h * HD, (h + 1) * HD) for h in heads])
        rows_v = np.concatenate([np.arange(3 * D + h * HD, 3 * D + (h + 1) * HD)
                                 for h in heads])
        rows_qku = np.concatenate([rows_q, rows_k, rows_u])
        b1_2d = np.ascontiguousarray(b1[rows_qku].reshape(6, P).T)
        in_maps_a.append(dict(
            xTr=xTrs[b],
            w1qr=_chunk_major(
                np.ascontiguousarray(w1[rows_qku].T).astype(nbf), 8),
            wvr=_chunk_major(
                np.ascontiguousarray(w1[rows_v].T).astype(nbf), 8),
            b1_2d=b1_2d,
            b1v_bc=np.ascontiguousarray(
                np.broadcast_to(b1[rows_v][None, :], (P, DC))).astype(nbf),
            inv128=inv128, ident=ident, strip=strip.astype(nbf),
            zeros_t=zeros_t, parts=parts_arr,
        ))
    res_a = run_bass_kernel_spmd(nca, in_maps_a, core_ids=list(range(8)),
                                 trace=trace)

    yT_full = [np.empty((D, S), nbf) for _ in range(B)]
    for c in range(8):
        b, hg = divmod(c, 4)
        yT_full[b][hg * DC:(hg + 1) * DC] = res_a.results[c]["yT_out"]

    # fold ln1's g1/beta1 and f2's b2 into the weights / residual
    w2g = w2 * g1[None, :]
    b2f = b2 + w2 @ beta1
    w2gT = np.ascontiguousarray(w2g.T).astype(nbf)
    wsr = np.ascontiguousarray(w2g.sum(axis=1)[None, :])
    gv2 = np.concatenate([g2.reshape(8, P).T, beta2.reshape(8, P).T], axis=1)
    gv2 = np.ascontiguousarray(gv2)
    ones_bm = np.ones((P, P), nbf)
    ones_rm = np.ones((P, P), np.float32)
    epsc = np.full((P, 1), EPS, np.float32)
    in_maps_b = []
    for c in range(8):
        b, qc = divmod(c, 4)
        sl = slice(qc * 512, (qc + 1) * 512)
        in_maps_b.append(dict(
            yr=_chunk_major(np.ascontiguousarray(yT_full[b][:, sl]), 8),
            w2r=_chunk_major(w2gT, 8),
            xbr=_chunk_major(
                np.ascontiguousarray(xT[b][:, sl] + b2f[:, None]), 8),
            wsr=wsr, gv2=gv2, ones_b=ones_bm, ones_r=ones_rm, epsc=epsc,
        ))
    res_b = run_bass_kernel_spmd(ncb, in_maps_b, core_ids=list(range(8)),
                                 trace=trace)

    out = np.empty((B, S, D), np.float32)
    for c in range(8):
        b, qc = divmod(c, 4)
        out[b, qc * 512:(qc + 1) * 512] = res_b.results[c]["outT"].T
    kernel.last_results = (res_a, res_b)
    return out


# revision 15
# speedup vs baseline: 1.2101x; 1.0052x over previous
"""HSTU block kernel for 8 TRN2 NeuronCores (Bass/Tile, bf16 matmuls).

Sharding: phase 1 (f1 + attention + u-gating) is data-parallel over batch
(B=2) x tensor-parallel over head groups (4 heads/core). Phase 2
(ln1 -> f2 -> +residual -> ln2) is row-parallel (512 rows/core). Host
gathers/reshards between the two launches; everything on-device is
feature-major so no transposes are ever needed.

Phase 2 exploits linearity: f2 runs on raw (un-normalized) y, with the
-mu correction folded in as a rank-1 matmul into the same PSUM
accumulation, so the GEMM overlaps the layernorm stats instead of
waiting on them.
"""
import os
import numpy as np
import ml_dtypes

import concourse.bacc as bacc
import concourse.mybir as mybir
from concourse.tile import TileContext
from concourse.bass_utils import run_bass_kernel_spmd

fp32 = mybir.dt.float32
fp32r = mybir.dt.float32r
bf16 = mybir.dt.bfloat16
AF = mybir.ActivationFunctionType
ALU = mybir.AluOpType
nbf = ml_dtypes.bfloat16

B, S, D, H, M = 2, 2048, 1024, 16, 4096
HD = D // H          # 64
EPS = 1e-5
P = 128
NB = S // P          # 16 seq blocks of 128
NG = S // 512        # 4 q-groups of 512
DC = 4 * HD          # 256 features per core in phase 1

_CACHE = {}


# ---------------------------------------------------------------- kernel A
def build_kernel_a(plan, jmin, n_strip, n_partial):
    """plan[g] = [(kt, lead, trail, cls4)], strip holds tiles jmin..jmin+n_strip-1."""
    nc = bacc.Bacc("TRN2", target_bir_lowering=False, debug=False, num_devices=8)

    xTr = nc.dram_tensor("xTr", [P, NG, 8, 512], bf16, kind="ExternalInput")
    w1qr = nc.dram_tensor("w1qr", [P, 6, 8, P], bf16, kind="ExternalInput")
    wvr = nc.dram_tensor("wvr", [P, 8, DC], bf16, kind="ExternalInput")
    b1_2d = nc.dram_tensor("b1_2d", [P, 6], fp32, kind="ExternalInput")
    b1v_bc = nc.dram_tensor("b1v_bc", [P, DC], bf16, kind="ExternalInput")
    inv128 = nc.dram_tensor("inv128", [P, P], bf16, kind="ExternalInput")
    ident = nc.dram_tensor("ident", [P, P], bf16, kind="ExternalInput")
    strip = nc.dram_tensor("strip", [P, n_strip * P], bf16, kind="ExternalInput")
    zeros_t = nc.dram_tensor("zeros_t", [P, 512], bf16, kind="ExternalInput")
    parts = nc.dram_tensor("parts", [P, max(n_partial, 1) * P], bf16,
                           kind="ExternalInput")
    yT_out = nc.dram_tensor("yT_out", [DC, S], bf16, kind="ExternalOutput")

    with TileContext(nc) as tc:
        with tc.tile_pool(name="const", bufs=1) as cpool, \
             tc.tile_pool(name="wpool", bufs=2) as wpool, \
             tc.tile_pool(name="big", bufs=1) as big, \
             tc.tile_pool(name="att", bufs=4) as apool, \
             tc.tile_pool(name="out", bufs=3) as opool, \
             tc.tile_pool(name="ps", bufs=1, space="PSUM") as ps:

            # few, large DMAs (one dma_start fans out over all 16 SDMA
            # engines; <=64KB transfers are descriptor-dominated)
            xs0 = wpool.tile([P, 8 * 512], bf16, name="xs", tag="xs", bufs=2)
            nc.sync.dma_start(xs0[:], xTr[:, 0])
            wq_all = big.tile([P, 8 * 768], bf16, name="wq_all", tag="wq")
            nc.sync.dma_start(wq_all[:, 0:1024], w1qr[:, 0])
            b1_sb = cpool.tile([P, 6], fp32, name="b1_sb")
            nc.sync.dma_start(b1_sb[:], b1_2d[:])
            nc.sync.dma_start(wq_all[:, 1024:6144], w1qr[:, 1:6])
            wv_all = big.tile([P, 8 * DC], bf16, name="wv_all", tag="wv")
            nc.sync.dma_start(wv_all[:], wvr[:])
            b1v_sb = cpool.tile([P, DC], bf16, name="b1v_sb")
            nc.sync.dma_start(b1v_sb[:], b1v_bc[:])
            inv128_sb = cpool.tile([P, P], bf16, name="inv128_sb")
            nc.sync.dma_start(inv128_sb[:], inv128[:])
            strip_sb = big.tile([P, n_strip * P], bf16, name="strip_sb",
                                tag="strip")
            nc.sync.dma_start(strip_sb[:], strip[:])
            ident_sb = cpool.tile([P, P], bf16, name="ident_sb")
            nc.sync.dma_start(ident_sb[:], ident[:])
            parts_all = cpool.tile([P, max(n_partial, 1) * P], bf16,
                                   name="parts_all")
            nc.sync.dma_start(parts_all[:], parts[:])
            zeros_sb = cpool.tile([P, 512], bf16, name="zeros_sb")
            nc.sync.dma_start(zeros_sb[:], zeros_t[:])

            # ---------------- f1: qku (feature-major) + v (natural) --------
            # k/v persist (attention history); q/u/x rotate per slab.
            # kh[p] stacks the head pair: rows 0:64 = head 2p, 64:128 = 2p+1.
            kh = [big.tile([P, S], bf16, name=f"khh{p}", tag=f"khh{p}")
                  for p in range(2)]
            v_sb = [big.tile([P, DC], bf16, name=f"v{sc}", tag=f"v{sc}")
                    for sc in range(NB)]

            def f1_slab(sg):
                if sg == 0:
                    xs = xs0
                else:
                    xs = wpool.tile([P, 8 * 512], bf16, name="xs", tag="xs",
                                    bufs=2)
                    nc.sync.dma_start(xs[:], xTr[:, sg])
                sl = slice(sg * 512, (sg + 1) * 512)
                # qs[p]: head pair stacked like kh
                qs = [wpool.tile([P, 512], bf16, name="qs", tag=f"qs{p}",
                                 bufs=2) for p in range(2)]
                us = [wpool.tile([P, 512], bf16, name="us", tag=f"us{p}",
                                 bufs=2) for p in range(2)]
                for fc in range(6):
                    pt = ps.tile([P, 512], fp32, name="f1ps", tag="f1mm",
                                 bufs=2)
                    for k in range(8):
                        nc.tensor.matmul(
                            pt[:],
                            wq_all[:, fc * 1024 + k * P:fc * 1024 + (k + 1) * P],
                            xs[:, k * 512:(k + 1) * 512],
                            start=(k == 0), stop=(k == 7))
                    if fc >= 4:
                        nc.scalar.activation(us[fc - 4][:], pt[:],
                                             AF.Silu, bias=b1_sb[:, fc:fc + 1],
                                             scale=1.0)
                    elif fc < 2:
                        nc.scalar.activation(qs[fc][:], pt[:],
                                             AF.Silu, bias=b1_sb[:, fc:fc + 1],
                                             scale=1.0)
                    else:
                        nc.scalar.activation(kh[fc - 2][:, sl], pt[:],
                                             AF.Silu, bias=b1_sb[:, fc:fc + 1],
                                             scale=1.0)
                for sc in range(4 * sg, 4 * sg + 4):
                    pt = ps.tile([P, DC], fp32, name="f1vps", tag="f1mm",
                                 bufs=2)
                    j = sc - 4 * sg
                    for k in range(8):
                        nc.tensor.matmul(
                            pt[:],
                            xs[:, k * 512 + j * P:k * 512 + (j + 1) * P],
                            wv_all[:, k * DC:(k + 1) * DC],
                            start=(k == 0), stop=False)
                    nc.tensor.matmul(pt[:], inv128_sb[:], b1v_sb[:],
                                     start=False, stop=True)
                    nc.scalar.activation(v_sb[sc][:], pt[:], AF.Silu, scale=1.0)
                return qs, us

            # ---------------- attention (per head pair) ----------------
            def attn_pair(p_idx, g, qs, us):
                kts = plan[g]
                gsl = slice(g * 512, (g + 1) * 512)
                avps = [ps.tile([64, 512], fp32, name="avp", tag=f"avph{hp}",
                                bufs=1) for hp in range(2)]
                first = True
                for ki, (kt, lead, trail, cls4) in enumerate(kts):
                    off, end = lead * P, 512 - trail * P
                    pe_bias = (ki % 4 == 0)
                    spp = ps.tile([P, 1024], fp32, name="sps", tag="spmm",
                                  bufs=2)
                    sb0 = (4 * g - kt + 15 - jmin) * P
                    for hp in range(2):
                        o2 = hp * 512
                        nc.tensor.matmul(
                            spp[:, o2 + off:o2 + end],
                            kh[p_idx][hp * 64:(hp + 1) * 64, kt * P:(kt + 1) * P],
                            qs[p_idx][hp * 64:(hp + 1) * 64, off:end],
                            start=True, stop=not pe_bias)
                        if pe_bias:
                            nc.tensor.matmul(spp[:, o2 + off:o2 + end],
                                             ident_sb[:],
                                             strip_sb[:, sb0 + off:sb0 + end],
                                             start=False, stop=True)
                    att = apool.tile([P, 1024], bf16, name="att",
                                     tag="att", bufs=6)
                    if first and off > 0:
                        for hp in range(2):
                            nc.vector.tensor_copy(att[:, hp * 512:hp * 512 + off],
                                                  zeros_sb[:, 0:off])
                    if first and end < 512:
                        for hp in range(2):
                            nc.vector.tensor_copy(
                                att[:, hp * 512 + end:(hp + 1) * 512],
                                zeros_sb[:, end:512])
                    if pe_bias:
                        if off == 0 and end == 512:
                            nc.scalar.activation(att[:], spp[:],
                                                 AF.Silu, scale=1.0)
                        else:
                            for hp in range(2):
                                o2 = hp * 512
                                nc.scalar.activation(att[:, o2 + off:o2 + end],
                                                     spp[:, o2 + off:o2 + end],
                                                     AF.Silu, scale=1.0)
                    else:
                        stmp = apool.tile([P, 1024], bf16, name="stmp",
                                          tag="stmp", bufs=3)
                        for hp in range(2):
                            o2 = hp * 512
                            nc.vector.tensor_tensor(
                                stmp[:, o2 + off:o2 + end],
                                spp[:, o2 + off:o2 + end],
                                strip_sb[:, sb0 + off:sb0 + end],
                                ALU.add)
                        if off == 0 and end == 512:
                            nc.scalar.activation(att[:], stmp[:],
                                                 AF.Silu, scale=1.0)
                        else:
                            for hp in range(2):
                                o2 = hp * 512
                                nc.scalar.activation(att[:, o2 + off:o2 + end],
                                                     stmp[:, o2 + off:o2 + end],
                                                     AF.Silu, scale=1.0)
                    for hp in range(2):
                        o2 = hp * 512
                        for j in range(lead, 4 - trail):
                            c = cls4[j]
                            if c == 0:
                                nc.vector.tensor_copy(
                                    att[:, o2 + j * P:o2 + (j + 1) * P],
                                    zeros_sb[:, 0:P])
                            elif c >= 2:
                                nc.gpsimd.tensor_tensor(
                                    att[:, o2 + j * P:o2 + (j + 1) * P],
                                    att[:, o2 + j * P:o2 + (j + 1) * P],
                                    parts_all[:, (c - 2) * P:(c - 1) * P],
                                    ALU.mult)
                    ao, ae = (0, 512) if first else (off, end)
                    for hp in range(2):
                        h = 2 * p_idx + hp
                        nc.tensor.matmul(
                            avps[hp][:, ao:ae],
                            v_sb[kt][:, h * 64:(h + 1) * 64],
                            att[:, hp * 512 + ao:hp * 512 + ae],
                            start=first, stop=(kt == kts[-1][0]))
                    first = False
                yg = opool.tile([P, 512], bf16, name="yg", tag="yg")
                if kts:
                    for hp in range(2):
                        nc.vector.tensor_tensor(
                            yg[hp * 64:(hp + 1) * 64, :], avps[hp][:],
                            us[p_idx][hp * 64:(hp + 1) * 64, :], ALU.mult)
                else:
                    nc.vector.tensor_copy(yg[:], zeros_sb[:])
                nc.gpsimd.dma_start(yT_out[p_idx * P:(p_idx + 1) * P, gsl],
                                    yg[:])

            for sg in range(NG):
                qs, us = f1_slab(sg)
                attn_pair(0, sg, qs, us)
                attn_pair(1, sg, qs, us)
    return nc


# ---------------------------------------------------------------- kernel B
def build_kernel_b():
    nc = bacc.Bacc("TRN2", target_bir_lowering=False, debug=False, num_devices=8)

    yr = nc.dram_tensor("yr", [P, 8, 512], bf16, kind="ExternalInput")
    w2r = nc.dram_tensor("w2r", [P, 8, D], bf16, kind="ExternalInput")
    xbr = nc.dram_tensor("xbr", [P, 8, 512], fp32, kind="ExternalInput")
    wsr = nc.dram_tensor("wsr", [1, D], fp32, kind="ExternalInput")
    gv2 = nc.dram_tensor("gv2", [P, 16], fp32, kind="ExternalInput")
    # gv2 cols: 0-7 g2, 8-15 beta2 (per 128-chunk)
    ones_b = nc.dram_tensor("ones_b", [P, P], bf16, kind="ExternalInput")
    ones_r = nc.dram_tensor("ones_r", [P, P], fp32, kind="ExternalInput")
    epsc = nc.dram_tensor("epsc", [P, 1], fp32, kind="ExternalInput")
    outT = nc.dram_tensor("outT", [D, 512], fp32, kind="ExternalOutput")

    with TileContext(nc) as tc:
        with tc.tile_pool(name="const", bufs=1) as cpool, \
             tc.tile_pool(name="big", bufs=1) as big, \
             tc.tile_pool(name="tmp", bufs=3) as tp, \
             tc.tile_pool(name="ps", bufs=1, space="PSUM") as ps:

            ytall = big.tile([P, 8 * 512], bf16, name="ytall", tag="yt")
            nc.sync.dma_start(ytall[:, 0:2048], yr[:, 0:4])
            nc.sync.dma_start(ytall[:, 2048:4096], yr[:, 4:8])
            onesb = cpool.tile([P, P], bf16, name="onesb")
            nc.sync.dma_start(onesb[:], ones_b[:])
            w2all = big.tile([P, 8 * D], bf16, name="w2all", tag="w2")
            nc.sync.dma_start(w2all[:], w2r[:])
            xball = big.tile([P, 8 * 512], fp32, name="xball", tag="xb")
            nc.sync.dma_start(xball[:], xbr[:])
            ws_sb = cpool.tile([1, D], fp32r, name="ws_sb")
            nc.sync.dma_start(ws_sb[:], wsr[:].bitcast(fp32r))
            onesr = cpool.tile([P, P], fp32r, name="onesr")
            nc.sync.dma_start(onesr[:], ones_r[:].bitcast(fp32r))
            gv = cpool.tile([P, 16], fp32, name="gv")
            nc.sync.dma_start(gv[:], gv2[:])
            eps_sb = cpool.tile([P, 1], fp32, name="eps_sb")
            nc.sync.dma_start(eps_sb[:], epsc[:])

            def yt(i):
                return ytall[:, i * 512:(i + 1) * 512]

            # ---- ln-a stats, broadcast across partitions via ones matmul
            psum_s = ps.tile([P, 512], fp32, name="lsa", tag="lsa")
            psum_q = ps.tile([P, 512], fp32, name="lqa", tag="lqa")
            for i in range(8):
                nc.tensor.matmul(psum_s[:], onesb[:], yt(i),
                                 start=(i == 0), stop=(i == 7))
            for i in range(8):
                sq = tp.tile([P, 512], bf16, name="sq", tag="sq")
                nc.scalar.activation(sq[:], yt(i), AF.Square, scale=1.0)
                nc.tensor.matmul(psum_q[:], onesb[:], sq[:],
                                 start=(i == 0), stop=(i == 7))
            negmu = tp.tile([1, 512], fp32r, name="negmu", tag="vr", bufs=2)
            nc.vector.tensor_scalar(negmu[:], psum_s[0:1, :], -1.0 / D, None,
                                    ALU.mult)
            mu_b = tp.tile([P, 512], fp32, name="mu_b", tag="mub", bufs=2)
            nc.vector.tensor_scalar(mu_b[:], psum_s[:], 1.0 / D, None, ALU.mult)
            t1 = tp.tile([P, 512], fp32, name="t1", tag="t1", bufs=2)
            nc.vector.tensor_tensor(t1[:], mu_b[:], mu_b[:], ALU.mult)
            msq = tp.tile([P, 512], fp32, name="msq", tag="msq", bufs=2)
            nc.vector.tensor_scalar(msq[:], psum_q[:], 1.0 / D, None, ALU.mult)
            var = tp.tile([P, 512], fp32, name="var", tag="var", bufs=2)
            nc.vector.tensor_tensor(var[:], msq[:], t1[:], ALU.subtract)
            sd_b = tp.tile([P, 512], fp32, name="sd_b", tag="sda", bufs=2)
            nc.scalar.activation(sd_b[:], var[:], AF.Sqrt,
                                 bias=eps_sb[:, 0:1], scale=1.0)
            rstd_b = tp.tile([P, 512], fp32, name="rstd_b", tag="rsa", bufs=2)
            nc.vector.reciprocal_approx_fast(rstd_b[:], sd_b[:])

            # ---- f2 on raw y; -mu correction folded in as rank-1 matmul
            t2r = [None] * 8
            for half in range(2):
                fcs = [4 * half + j for j in range(4)]
                pts = {fc: ps.tile([P, 512], fp32, name=f"f2ps{fc}",
                                   tag=f"f2ps{fc % 4}", bufs=1) for fc in fcs}
                for k in range(8):
                    for fc in fcs:
                        nc.tensor.matmul(
                            pts[fc][:],
                            w2all[:, k * D + fc * P:k * D + (fc + 1) * P],
                            yt(k), start=(k == 0), stop=False)
                for fc in fcs:
                    nc.tensor.matmul(pts[fc][:],
                                     ws_sb[0:1, fc * P:(fc + 1) * P],
                                     negmu[:], start=False, stop=True)
                    t = big.tile([P, 512], fp32r, name=f"t2r{fc}",
                                 tag=f"t2r{fc}")
                    nc.vector.tensor_tensor(t[:], pts[fc][:], rstd_b[:],
                                            ALU.mult)
                    nc.vector.tensor_tensor(
                        t[:], t[:], xball[:, fc * 512:(fc + 1) * 512], ALU.add)
                    t2r[fc] = t

            # ---- ln-b stats (broadcast) + apply
            psum2_s = ps.tile([P, 512], fp32, name="lsb", tag="lsb")
            psum2_q = ps.tile([P, 512], fp32, name="lqb", tag="lqb")
            for i in range(8):
                nc.tensor.matmul(psum2_s[:], onesr[:], t2r[i][:],
                                 start=(i == 0), stop=(i == 7))
            for i in range(8):
                sq2 = tp.tile([P, 512], fp32r, name="sq2", tag="sq")
                nc.scalar.activation(sq2[:], t2r[i][:].bitcast(fp32),
                                     AF.Square, scale=1.0)
                nc.tensor.matmul(psum2_q[:], onesr[:], sq2[:],
                                 start=(i == 0), stop=(i == 7))
            mu2 = tp.tile([P, 512], fp32, name="mu2", tag="mub", bufs=2)
            nc.vector.tensor_scalar(mu2[:], psum2_s[:], 1.0 / D, None, ALU.mult)
            t12 = tp.tile([P, 512], fp32, name="t12", tag="t1", bufs=2)
            nc.vector.tensor_tensor(t12[:], mu2[:], mu2[:], ALU.mult)
            msq2 = tp.tile([P, 512], fp32, name="msq2", tag="msq", bufs=2)
            nc.vector.tensor_scalar(msq2[:], psum2_q[:], 1.0 / D, None, ALU.mult)
            var2 = tp.tile([P, 512], fp32, name="var2", tag="var", bufs=2)
            nc.vector.tensor_tensor(var2[:], msq2[:], t12[:], ALU.subtract)
            sd2 = tp.tile([P, 512], fp32, name="sd2", tag="sda", bufs=2)
            nc.scalar.activation(sd2[:], var2[:], AF.Sqrt,
                                 bias=eps_sb[:, 0:1], scale=1.0)
            rstd2 = tp.tile([P, 512], fp32, name="rstd2", tag="rsa", bufs=2)
            nc.vector.reciprocal_approx_fast(rstd2[:], sd2[:])
            nb2 = tp.tile([P, 512], fp32, name="nb2", tag="nb2")
            nc.vector.tensor_tensor(nb2[:], mu2[:], rstd2[:], ALU.mult)

            for i in range(8):
                # split the serial apply tail across DVE and the idle GpSimd
                eng = nc.gpsimd if i >= 5 else nc.vector
                t = tp.tile([P, 512], fp32, name="lnt", tag="lnt")
                eng.tensor_tensor(t[:], t2r[i][:].bitcast(fp32),
                                  rstd2[:], ALU.mult)
                eng.tensor_tensor(t[:], t[:], nb2[:], ALU.subtract)
                o = tp.tile([P, 512], fp32, name="lno", tag=f"lno{i}", bufs=1)
                eng.tensor_scalar(o[:], t[:], gv[:, i:i + 1],
                                  gv[:, 8 + i:8 + i + 1],
                                  ALU.mult, ALU.add)
                nc.sync.dma_start(outT[i * P:(i + 1) * P, :], o[:])
    return nc


# ---------------------------------------------------------------- host side
def _classify_mask(mask):
    keep = (mask.reshape(S, S) >= 0)
    block_cls = [[0] * NB for _ in range(NB)]  # [kt][qb]
    partials = []
    pmap = {}
    for kt in range(NB):
        for qb in range(NB):
            sub = keep[qb * P:(qb + 1) * P, kt * P:(kt + 1) * P]
            if sub.all():
                block_cls[kt][qb] = 1
            elif not sub.any():
                block_cls[kt][qb] = 0
            else:
                tile = np.ascontiguousarray(sub.T.astype(np.float32))
                key = tile.tobytes()
                if key not in pmap:
                    pmap[key] = len(partials)
                    partials.append(tile)
                block_cls[kt][qb] = 2 + pmap[key]
    return block_cls, partials


def _plan_attn(block_cls):
    """Per (g, kt): lead/trail skip blocks + strip tile range used."""
    plan = {}
    jmin, jmax = 31, 0
    for g in range(NG):
        kts = []
        for kt in range(NB):
            cls4 = [block_cls[kt][4 * g + j] for j in range(4)]
            if all(c == 0 for c in cls4):
                continue
            lead = 0
            while cls4[lead] == 0:
                lead += 1
            trail = 0
            while cls4[3 - trail] == 0:
                trail += 1
            jj0 = 4 * g - kt + 15
            jmin = min(jmin, jj0 + lead)
            jmax = max(jmax, jj0 + 3 - trail)
            kts.append((kt, lead, trail, cls4))
        plan[g] = kts
    if jmin > jmax:
        jmin, jmax = 0, 0
    return plan, jmin, jmax


def _get_compiled(mask_bytes, mask):
    if mask_bytes in _CACHE:
        return _CACHE[mask_bytes]
    block_cls, partials = _classify_mask(mask)
    plan, jmin, jmax = _plan_attn(block_cls)
    n_strip = jmax - jmin + 1
    nca = build_kernel_a(plan, jmin, n_strip, len(partials))
    nca.compile()
    ncb = build_kernel_b()
    ncb.compile()
    _CACHE[mask_bytes] = (nca, ncb, jmin, n_strip, partials)
    return _CACHE[mask_bytes]


def _chunk_major(a, nk):
    """[nk*128, F] -> [128, nk, F] (partition-major chunk layout)."""
    f = a.shape[1]
    return np.ascontiguousarray(a.reshape(nk, P, f).transpose(1, 0, 2))


def kernel(x, mask, w1, b1, w2, b2, g1, beta1, g2, beta2, pos_w):
    x = np.asarray(x, np.float32)
    w1 = np.asarray(w1, np.float32)
    b1 = np.asarray(b1, np.float32)
    w2 = np.asarray(w2, np.float32)
    b2 = np.asarray(b2, np.float32)
    g1 = np.asarray(g1, np.float32)
    beta1 = np.asarray(beta1, np.float32)
    g2 = np.asarray(g2, np.float32)
    beta2 = np.asarray(beta2, np.float32)
    pos_w = np.asarray(pos_w, np.float32)
    mask_np = np.asarray(mask)

    nca, ncb, jmin, n_strip, partials = _get_compiled(mask_np.tobytes(), mask_np)

    trace = bool(int(os.environ.get("HSTU_TRACE", "0")))
    strip = np.zeros((P, n_strip * P), np.float32)
    pidx = np.arange(P)[:, None]
    fidx = np.arange(P)[None, :]
    for i in range(n_strip):
        jj = jmin + i
        base = M - 1 - P * (jj - 15)
        strip[:, i * P:(i + 1) * P] = pos_w[base + pidx - fidx]
    ident = np.eye(P, dtype=nbf)
    inv128 = np.full((P, P), 1.0 / P, nbf)
    zeros_t = np.zeros((P, 512), nbf)
    parts_arr = (np.concatenate(partials, axis=1).astype(nbf) if partials
                 else np.zeros((P, P), nbf))

    xT = [np.ascontiguousarray(x[b].T) for b in range(B)]
    # [128, NG, 8, 512]: slab-major then k-chunk so one DMA covers a slab
    xTrs = [np.ascontiguousarray(
        t.astype(nbf).reshape(8, P, NG, 512).transpose(1, 2, 0, 3))
        for t in xT]
    in_maps_a = []
    for c in range(8):
        b, hg = divmod(c, 4)
        heads = [4 * hg + i for i in range(4)]
        rows_q = np.concatenate([np.arange(D + h * HD, D + (h + 1) * HD)
                                 for h in heads])
        rows_k = np.concatenate([np.arange(2 * D + h * HD, 2 * D + (h + 1) * HD)
                                 for h in heads])
        rows_u = np.concatenate([np.arange(h * HD, (h + 1) * HD) for h in heads])
        rows_v = np.concatenate([np.arange(3 * D + h * HD, 3 * D + (h + 1) * HD)
                                 for h in heads])
        rows_qku = np.concatenate([rows_q, rows_k, rows_u])
        b1_2d = np.ascontiguousarray(b1[rows_qku].reshape(6, P).T)
        in_maps_a.append(dict(
            xTr=xTrs[b],
            w1qr=np.ascontiguousarray(
                w1[rows_qku].T.astype(nbf).reshape(8, P, 6, P)
                .transpose(1, 2, 0, 3)),
            wvr=_chunk_major(
                np.ascontiguousarray(w1[rows_v].T).astype(nbf), 8),
            b1_2d=b1_2d,
            b1v_bc=np.ascontiguousarray(
                np.broadcast_to(b1[rows_v][None, :], (P, DC))).astype(nbf),
            inv128=inv128, ident=ident, strip=strip.astype(nbf),
            zeros_t=zeros_t, parts=parts_arr,
        ))
    res_a = run_bass_kernel_spmd(nca, in_maps_a, core_ids=list(range(8)),
                                 trace=trace)

    yT_full = [np.empty((D, S), nbf) for _ in range(B)]
    for c in range(8):
        b, hg = divmod(c, 4)
        yT_full[b][hg * DC:(hg + 1) * DC] = res_a.results[c]["yT_out"]

    # fold ln1's g1/beta1 and f2's b2 into the weights / residual
    w2g = w2 * g1[None, :]
    b2f = b2 + w2 @ beta1
    w2gT = np.ascontiguousarray(w2g.T).astype(nbf)
    wsr = np.ascontiguousarray(w2g.sum(axis=1)[None, :])
    gv2 = np.concatenate([g2.reshape(8, P).T, beta2.reshape(8, P).T], axis=1)
    gv2 = np.ascontiguousarray(gv2)
    ones_bm = np.ones((P, P), nbf)
    ones_rm = np.ones((P, P), np.float32)
    epsc = np.full((P, 1), EPS, np.float32)
    in_maps_b = []
    for c in range(8):
        b, qc = divmod(c, 4)
        sl = slice(qc * 512, (qc + 1) * 512)
        in_maps_b.append(dict(
            yr=_chunk_major(np.ascontiguousarray(yT_full[b][:, sl]), 8),
            w2r=_chunk_major(w2gT, 8),
            xbr=_chunk_major(
                np.ascontiguousarray(xT[b][:, sl] + b2f[:, None]), 8),
            wsr=wsr, gv2=gv2, ones_b=ones_bm, ones_r=ones_rm, epsc=epsc,
        ))
    res_b = run_bass_kernel_spmd(ncb, in_maps_b, core_ids=list(range(8)),
                                 trace=trace)

    out = np.empty((B, S, D), np.float32)
    for c in range(8):
        b, qc = divmod(c, 4)
        out[b, qc * 512:(qc + 1) * 512] = res_b.results[c]["outT"].T
    kernel.last_results = (res_a, res_b)
    return out


# revision 16
# speedup vs baseline: 1.2658x; 1.0461x over previous
"""HSTU block kernel for 8 TRN2 NeuronCores (Bass/Tile, bf16 matmuls).

Sharding: phase 1 (f1 + attention + u-gating) is data-parallel over batch
(B=2) x tensor-parallel over head groups (4 heads/core). Phase 2
(ln1 -> f2 -> +residual -> ln2) is row-parallel (512 rows/core). Host
gathers/reshards between the two launches; everything on-device is
feature-major so no transposes are ever needed.

Phase 2 exploits linearity: f2 runs on raw (un-normalized) y, with the
-mu correction folded in as a rank-1 matmul into the same PSUM
accumulation, so the GEMM overlaps the layernorm stats instead of
waiting on them.
"""
import os
import numpy as np
import ml_dtypes

import concourse.bacc as bacc
import concourse.mybir as mybir
from concourse.tile import TileContext
from concourse.bass_utils import run_bass_kernel_spmd

fp32 = mybir.dt.float32
fp32r = mybir.dt.float32r
bf16 = mybir.dt.bfloat16
AF = mybir.ActivationFunctionType
ALU = mybir.AluOpType
nbf = ml_dtypes.bfloat16

B, S, D, H, M = 2, 2048, 1024, 16, 4096
HD = D // H          # 64
EPS = 1e-5
P = 128
NB = S // P          # 16 seq blocks of 128
NG = S // 512        # 4 q-groups of 512
DC = 4 * HD          # 256 features per core in phase 1

_CACHE = {}


# ---------------------------------------------------------------- kernel A
def build_kernel_a(plan, jmin, n_strip, n_partial, b1_zero):
    """plan[g] = [(kt, lead, trail, cls4)], strip holds tiles jmin..jmin+n_strip-1."""
    nc = bacc.Bacc("TRN2", target_bir_lowering=False, debug=False, num_devices=8)

    xTr = nc.dram_tensor("xTr", [P, NG, 8, 512], bf16, kind="ExternalInput")
    w1qr = nc.dram_tensor("w1qr", [P, 6, 8, P], bf16, kind="ExternalInput")
    wvr = nc.dram_tensor("wvr", [P, 8, DC], bf16, kind="ExternalInput")
    b1_2d = nc.dram_tensor("b1_2d", [P, 6], fp32, kind="ExternalInput")
    b1v_bc = nc.dram_tensor("b1v_bc", [P, DC], bf16, kind="ExternalInput")
    inv128 = nc.dram_tensor("inv128", [P, P], bf16, kind="ExternalInput")
    ident = nc.dram_tensor("ident", [P, P], bf16, kind="ExternalInput")
    strip = nc.dram_tensor("strip", [P, n_strip * P], bf16, kind="ExternalInput")
    zeros_t = nc.dram_tensor("zeros_t", [P, 512], bf16, kind="ExternalInput")
    parts = nc.dram_tensor("parts", [P, max(n_partial, 1) * P], bf16,
                           kind="ExternalInput")
    yT_out = nc.dram_tensor("yT_out", [DC, S], bf16, kind="ExternalOutput")

    with TileContext(nc) as tc:
        with tc.tile_pool(name="const", bufs=1) as cpool, \
             tc.tile_pool(name="wpool", bufs=2) as wpool, \
             tc.tile_pool(name="big", bufs=1) as big, \
             tc.tile_pool(name="att", bufs=4) as apool, \
             tc.tile_pool(name="out", bufs=3) as opool, \
             tc.tile_pool(name="ps", bufs=1, space="PSUM") as ps:

            # few, large DMAs (one dma_start fans out over all 16 SDMA
            # engines; <=64KB transfers are descriptor-dominated)
            xs0 = wpool.tile([P, 8 * 512], bf16, name="xs", tag="xs", bufs=2)
            nc.sync.dma_start(xs0[:], xTr[:, 0])
            wq_all = big.tile([P, 8 * 768], bf16, name="wq_all", tag="wq")
            nc.sync.dma_start(wq_all[:, 0:1024], w1qr[:, 0])
            b1_sb = cpool.tile([P, 6], fp32, name="b1_sb")
            nc.sync.dma_start(b1_sb[:], b1_2d[:])
            nc.sync.dma_start(wq_all[:, 1024:6144], w1qr[:, 1:6])
            wv_all = big.tile([P, 8 * DC], bf16, name="wv_all", tag="wv")
            nc.sync.dma_start(wv_all[:], wvr[:])
            b1v_sb = cpool.tile([P, DC], bf16, name="b1v_sb")
            nc.sync.dma_start(b1v_sb[:], b1v_bc[:])
            inv128_sb = cpool.tile([P, P], bf16, name="inv128_sb")
            nc.sync.dma_start(inv128_sb[:], inv128[:])
            strip_sb = big.tile([P, n_strip * P], bf16, name="strip_sb",
                                tag="strip")
            nc.sync.dma_start(strip_sb[:], strip[:])
            ident_sb = cpool.tile([P, P], bf16, name="ident_sb")
            nc.sync.dma_start(ident_sb[:], ident[:])
            parts_all = cpool.tile([P, max(n_partial, 1) * P], bf16,
                                   name="parts_all")
            nc.sync.dma_start(parts_all[:], parts[:])
            zeros_sb = cpool.tile([P, 512], bf16, name="zeros_sb")
            nc.sync.dma_start(zeros_sb[:], zeros_t[:])

            # ---------------- f1: qku (feature-major) + v (natural) --------
            # k/v persist (attention history); q/u/x rotate per slab.
            # kh[p] stacks the head pair: rows 0:64 = head 2p, 64:128 = 2p+1.
            kh = [big.tile([P, S], bf16, name=f"khh{p}", tag=f"khh{p}")
                  for p in range(2)]
            v_sb = [big.tile([P, DC], bf16, name=f"v{sc}", tag=f"v{sc}")
                    for sc in range(NB)]

            def f1_slab(sg):
                if sg == 0:
                    xs = xs0
                else:
                    xs = wpool.tile([P, 8 * 512], bf16, name="xs", tag="xs",
                                    bufs=2)
                    nc.sync.dma_start(xs[:], xTr[:, sg])
                sl = slice(sg * 512, (sg + 1) * 512)
                # qs[p]: head pair stacked like kh
                qs = [wpool.tile([P, 512], bf16, name="qs", tag=f"qs{p}",
                                 bufs=2) for p in range(2)]
                us = [wpool.tile([P, 512], bf16, name="us", tag=f"us{p}",
                                 bufs=2) for p in range(2)]
                for fc in range(6):
                    pt = ps.tile([P, 512], fp32, name="f1ps", tag="f1mm",
                                 bufs=2)
                    for k in range(8):
                        nc.tensor.matmul(
                            pt[:],
                            wq_all[:, fc * 1024 + k * P:fc * 1024 + (k + 1) * P],
                            xs[:, k * 512:(k + 1) * 512],
                            start=(k == 0), stop=(k == 7))
                    if fc >= 4:
                        nc.scalar.activation(us[fc - 4][:], pt[:],
                                             AF.Silu, bias=b1_sb[:, fc:fc + 1],
                                             scale=1.0)
                    elif fc < 2:
                        nc.scalar.activation(qs[fc][:], pt[:],
                                             AF.Silu, bias=b1_sb[:, fc:fc + 1],
                                             scale=1.0)
                    else:
                        nc.scalar.activation(kh[fc - 2][:, sl], pt[:],
                                             AF.Silu, bias=b1_sb[:, fc:fc + 1],
                                             scale=1.0)
                for sc in range(4 * sg, 4 * sg + 4):
                    pt = ps.tile([P, DC], fp32, name="f1vps", tag="f1mm",
                                 bufs=2)
                    j = sc - 4 * sg
                    for k in range(8):
                        nc.tensor.matmul(
                            pt[:],
                            xs[:, k * 512 + j * P:k * 512 + (j + 1) * P],
                            wv_all[:, k * DC:(k + 1) * DC],
                            start=(k == 0), stop=(b1_zero and k == 7))
                    if not b1_zero:
                        nc.tensor.matmul(pt[:], inv128_sb[:], b1v_sb[:],
                                         start=False, stop=True)
                    nc.scalar.activation(v_sb[sc][:], pt[:], AF.Silu, scale=1.0)
                return qs, us

            # ---------------- attention (per head pair) ----------------
            def attn_pair(p_idx, g, qs, us):
                kts = plan[g]
                gsl = slice(g * 512, (g + 1) * 512)
                avps = [ps.tile([64, 512], fp32, name="avp", tag=f"avph{hp}",
                                bufs=1) for hp in range(2)]
                first = True
                for ki, (kt, lead, trail, cls4) in enumerate(kts):
                    off, end = lead * P, 512 - trail * P
                    # early groups: PE is saturated by f1 -> bias on DVE;
                    # late groups: no f1 left, PE idles -> bias on PE
                    if g <= 1:
                        pe_bias = False
                    elif g == 2:
                        pe_bias = (ki % 3 == 0)
                    else:
                        pe_bias = (ki % 2 == 0)
                    spp = ps.tile([P, 1024], fp32, name="sps", tag="spmm",
                                  bufs=2)
                    sb0 = (4 * g - kt + 15 - jmin) * P
                    for hp in range(2):
                        o2 = hp * 512
                        nc.tensor.matmul(
                            spp[:, o2 + off:o2 + end],
                            kh[p_idx][hp * 64:(hp + 1) * 64, kt * P:(kt + 1) * P],
                            qs[p_idx][hp * 64:(hp + 1) * 64, off:end],
                            start=True, stop=not pe_bias)
                        if pe_bias:
                            nc.tensor.matmul(spp[:, o2 + off:o2 + end],
                                             ident_sb[:],
                                             strip_sb[:, sb0 + off:sb0 + end],
                                             start=False, stop=True)
                    att = apool.tile([P, 1024], bf16, name="att",
                                     tag="att", bufs=6)
                    if first and off > 0:
                        for hp in range(2):
                            nc.vector.tensor_copy(att[:, hp * 512:hp * 512 + off],
                                                  zeros_sb[:, 0:off])
                    if first and end < 512:
                        for hp in range(2):
                            nc.vector.tensor_copy(
                                att[:, hp * 512 + end:(hp + 1) * 512],
                                zeros_sb[:, end:512])
                    if pe_bias:
                        if off == 0 and end == 512:
                            nc.scalar.activation(att[:], spp[:],
                                                 AF.Silu, scale=1.0)
                        else:
                            for hp in range(2):
                                o2 = hp * 512
                                nc.scalar.activation(att[:, o2 + off:o2 + end],
                                                     spp[:, o2 + off:o2 + end],
                                                     AF.Silu, scale=1.0)
                    else:
                        stmp = apool.tile([P, 1024], bf16, name="stmp",
                                          tag="stmp", bufs=3)
                        for hp in range(2):
                            o2 = hp * 512
                            nc.vector.tensor_tensor(
                                stmp[:, o2 + off:o2 + end],
                                spp[:, o2 + off:o2 + end],
                                strip_sb[:, sb0 + off:sb0 + end],
                                ALU.add)
                        if off == 0 and end == 512:
                            nc.scalar.activation(att[:], stmp[:],
                                                 AF.Silu, scale=1.0)
                        else:
                            for hp in range(2):
                                o2 = hp * 512
                                nc.scalar.activation(att[:, o2 + off:o2 + end],
                                                     stmp[:, o2 + off:o2 + end],
                                                     AF.Silu, scale=1.0)
                    for hp in range(2):
                        o2 = hp * 512
                        for j in range(lead, 4 - trail):
                            c = cls4[j]
                            if c == 0:
                                nc.vector.tensor_copy(
                                    att[:, o2 + j * P:o2 + (j + 1) * P],
                                    zeros_sb[:, 0:P])
                            elif c >= 2:
                                nc.gpsimd.tensor_tensor(
                                    att[:, o2 + j * P:o2 + (j + 1) * P],
                                    att[:, o2 + j * P:o2 + (j + 1) * P],
                                    parts_all[:, (c - 2) * P:(c - 1) * P],
                                    ALU.mult)
                    ao, ae = (0, 512) if first else (off, end)
                    for hp in range(2):
                        h = 2 * p_idx + hp
                        nc.tensor.matmul(
                            avps[hp][:, ao:ae],
                            v_sb[kt][:, h * 64:(h + 1) * 64],
                            att[:, hp * 512 + ao:hp * 512 + ae],
                            start=first, stop=(kt == kts[-1][0]))
                    first = False
                yg = opool.tile([P, 512], bf16, name="yg", tag="yg")
                if kts:
                    for hp in range(2):
                        nc.vector.tensor_tensor(
                            yg[hp * 64:(hp + 1) * 64, :], avps[hp][:],
                            us[p_idx][hp * 64:(hp + 1) * 64, :], ALU.mult)
                else:
                    nc.vector.tensor_copy(yg[:], zeros_sb[:])
                nc.gpsimd.dma_start(yT_out[p_idx * P:(p_idx + 1) * P, gsl],
                                    yg[:])

            for sg in range(NG):
                qs, us = f1_slab(sg)
                attn_pair(0, sg, qs, us)
                attn_pair(1, sg, qs, us)
    return nc


# ---------------------------------------------------------------- kernel B
def build_kernel_b(g2_trivial):
    nc = bacc.Bacc("TRN2", target_bir_lowering=False, debug=False, num_devices=8)

    yr = nc.dram_tensor("yr", [P, 8, 512], bf16, kind="ExternalInput")
    w2r = nc.dram_tensor("w2r", [P, 8, D], bf16, kind="ExternalInput")
    xbr = nc.dram_tensor("xbr", [P, 8, 512], fp32, kind="ExternalInput")
    wsr = nc.dram_tensor("wsr", [1, D], fp32, kind="ExternalInput")
    gv2 = nc.dram_tensor("gv2", [P, 16], fp32, kind="ExternalInput")
    # gv2 cols: 0-7 g2, 8-15 beta2 (per 128-chunk)
    ones_b = nc.dram_tensor("ones_b", [P, P], bf16, kind="ExternalInput")
    ones_r = nc.dram_tensor("ones_r", [P, P], fp32, kind="ExternalInput")
    epsc = nc.dram_tensor("epsc", [P, 1], fp32, kind="ExternalInput")
    outT = nc.dram_tensor("outT", [D, 512], fp32, kind="ExternalOutput")

    with TileContext(nc) as tc:
        with tc.tile_pool(name="const", bufs=1) as cpool, \
             tc.tile_pool(name="big", bufs=1) as big, \
             tc.tile_pool(name="tmp", bufs=3) as tp, \
             tc.tile_pool(name="ps", bufs=1, space="PSUM") as ps:

            ytall = big.tile([P, 8 * 512], bf16, name="ytall", tag="yt")
            nc.sync.dma_start(ytall[:, 0:2048], yr[:, 0:4])
            nc.sync.dma_start(ytall[:, 2048:4096], yr[:, 4:8])
            onesb = cpool.tile([P, P], bf16, name="onesb")
            nc.sync.dma_start(onesb[:], ones_b[:])
            w2all = big.tile([P, 8 * D], bf16, name="w2all", tag="w2")
            nc.sync.dma_start(w2all[:], w2r[:])
            xball = big.tile([P, 8 * 512], fp32, name="xball", tag="xb")
            nc.sync.dma_start(xball[:], xbr[:])
            ws_sb = cpool.tile([1, D], fp32r, name="ws_sb")
            nc.sync.dma_start(ws_sb[:], wsr[:].bitcast(fp32r))
            onesr = cpool.tile([P, P], fp32r, name="onesr")
            nc.sync.dma_start(onesr[:], ones_r[:].bitcast(fp32r))
            gv = cpool.tile([P, 16], fp32, name="gv")
            nc.sync.dma_start(gv[:], gv2[:])
            eps_sb = cpool.tile([P, 1], fp32, name="eps_sb")
            nc.sync.dma_start(eps_sb[:], epsc[:])

            def yt(i):
                return ytall[:, i * 512:(i + 1) * 512]

            # ---- ln-a stats, broadcast across partitions via ones matmul
            psum_s = ps.tile([P, 512], fp32, name="lsa", tag="lsa")
            psum_q = ps.tile([P, 512], fp32, name="lqa", tag="lqa")
            for i in range(8):
                nc.tensor.matmul(psum_s[:], onesb[:], yt(i),
                                 start=(i == 0), stop=(i == 7))
            for i in range(8):
                sq = tp.tile([P, 512], bf16, name="sq", tag="sq")
                nc.scalar.activation(sq[:], yt(i), AF.Square, scale=1.0)
                nc.tensor.matmul(psum_q[:], onesb[:], sq[:],
                                 start=(i == 0), stop=(i == 7))
            negmu = tp.tile([1, 512], fp32r, name="negmu", tag="vr", bufs=2)
            nc.vector.tensor_scalar(negmu[:], psum_s[0:1, :], -1.0 / D, None,
                                    ALU.mult)
            mu_b = tp.tile([P, 512], fp32, name="mu_b", tag="mub", bufs=2)
            nc.vector.tensor_scalar(mu_b[:], psum_s[:], 1.0 / D, None, ALU.mult)
            t1 = tp.tile([P, 512], fp32, name="t1", tag="t1", bufs=2)
            nc.vector.tensor_tensor(t1[:], mu_b[:], mu_b[:], ALU.mult)
            msq = tp.tile([P, 512], fp32, name="msq", tag="msq", bufs=2)
            nc.vector.tensor_scalar(msq[:], psum_q[:], 1.0 / D, None, ALU.mult)
            var = tp.tile([P, 512], fp32, name="var", tag="var", bufs=2)
            nc.vector.tensor_tensor(var[:], msq[:], t1[:], ALU.subtract)
            sd_b = tp.tile([P, 512], fp32, name="sd_b", tag="sda", bufs=2)
            nc.scalar.activation(sd_b[:], var[:], AF.Sqrt,
                                 bias=eps_sb[:, 0:1], scale=1.0)
            rstd_b = tp.tile([P, 512], fp32, name="rstd_b", tag="rsa", bufs=2)
            nc.vector.reciprocal_approx_fast(rstd_b[:], sd_b[:])

            # ---- f2 on raw y; -mu correction folded in as rank-1 matmul
            t2r = [None] * 8
            for half in range(2):
                fcs = [4 * half + j for j in range(4)]
                pts = {fc: ps.tile([P, 512], fp32, name=f"f2ps{fc}",
                                   tag=f"f2ps{fc % 4}", bufs=1) for fc in fcs}
                for k in range(8):
                    for fc in fcs:
                        nc.tensor.matmul(
                            pts[fc][:],
                            w2all[:, k * D + fc * P:k * D + (fc + 1) * P],
                            yt(k), start=(k == 0), stop=False)
                for fc in fcs:
                    nc.tensor.matmul(pts[fc][:],
                                     ws_sb[0:1, fc * P:(fc + 1) * P],
                                     negmu[:], start=False, stop=True)
                    t = big.tile([P, 512], fp32r, name=f"t2r{fc}",
                                 tag=f"t2r{fc}")
                    nc.vector.tensor_tensor(t[:], pts[fc][:], rstd_b[:],
                                            ALU.mult)
                    nc.vector.tensor_tensor(
                        t[:], t[:], xball[:, fc * 512:(fc + 1) * 512], ALU.add)
                    t2r[fc] = t

            # ---- ln-b stats (broadcast) + apply
            psum2_s = ps.tile([P, 512], fp32, name="lsb", tag="lsb")
            psum2_q = ps.tile([P, 512], fp32, name="lqb", tag="lqb")
            for i in range(8):
                nc.tensor.matmul(psum2_s[:], onesr[:], t2r[i][:],
                                 start=(i == 0), stop=(i == 7))
            for i in range(8):
                sq2 = tp.tile([P, 512], fp32r, name="sq2", tag="sq")
                nc.scalar.activation(sq2[:], t2r[i][:].bitcast(fp32),
                                     AF.Square, scale=1.0)
                nc.tensor.matmul(psum2_q[:], onesr[:], sq2[:],
                                 start=(i == 0), stop=(i == 7))
            mu2 = tp.tile([P, 512], fp32, name="mu2", tag="mub", bufs=2)
            nc.vector.tensor_scalar(mu2[:], psum2_s[:], 1.0 / D, None, ALU.mult)
            t12 = tp.tile([P, 512], fp32, name="t12", tag="t1", bufs=2)
            nc.vector.tensor_tensor(t12[:], mu2[:], mu2[:], ALU.mult)
            msq2 = tp.tile([P, 512], fp32, name="msq2", tag="msq", bufs=2)
            nc.vector.tensor_scalar(msq2[:], psum2_q[:], 1.0 / D, None, ALU.mult)
            var2 = tp.tile([P, 512], fp32, name="var2", tag="var", bufs=2)
            nc.vector.tensor_tensor(var2[:], msq2[:], t12[:], ALU.subtract)
            sd2 = tp.tile([P, 512], fp32, name="sd2", tag="sda", bufs=2)
            nc.scalar.activation(sd2[:], var2[:], AF.Sqrt,
                                 bias=eps_sb[:, 0:1], scale=1.0)
            rstd2 = tp.tile([P, 512], fp32, name="rstd2", tag="rsa", bufs=2)
            nc.vector.reciprocal_approx_fast(rstd2[:], sd2[:])
            nb2 = tp.tile([P, 512], fp32, name="nb2", tag="nb2")
            nc.vector.tensor_tensor(nb2[:], mu2[:], rstd2[:], ALU.mult)

            for i in range(8):
                t = tp.tile([P, 512], fp32, name="lnt", tag="lnt")
                nc.vector.tensor_tensor(t[:], t2r[i][:].bitcast(fp32),
                                        rstd2[:], ALU.mult)
                if g2_trivial:
                    o = tp.tile([P, 512], fp32, name="lno", tag=f"lno{i}",
                                bufs=1)
                    nc.vector.tensor_tensor(o[:], t[:], nb2[:], ALU.subtract)
                else:
                    nc.vector.tensor_tensor(t[:], t[:], nb2[:], ALU.subtract)
                    o = tp.tile([P, 512], fp32, name="lno", tag=f"lno{i}",
                                bufs=1)
                    nc.vector.tensor_scalar(o[:], t[:], gv[:, i:i + 1],
                                            gv[:, 8 + i:8 + i + 1],
                                            ALU.mult, ALU.add)
                nc.sync.dma_start(outT[i * P:(i + 1) * P, :], o[:])
    return nc


# ---------------------------------------------------------------- host side
def _classify_mask(mask):
    keep = (mask.reshape(S, S) >= 0)
    block_cls = [[0] * NB for _ in range(NB)]  # [kt][qb]
    partials = []
    pmap = {}
    for kt in range(NB):
        for qb in range(NB):
            sub = keep[qb * P:(qb + 1) * P, kt * P:(kt + 1) * P]
            if sub.all():
                block_cls[kt][qb] = 1
            elif not sub.any():
                block_cls[kt][qb] = 0
            else:
                tile = np.ascontiguousarray(sub.T.astype(np.float32))
                key = tile.tobytes()
                if key not in pmap:
                    pmap[key] = len(partials)
                    partials.append(tile)
                block_cls[kt][qb] = 2 + pmap[key]
    return block_cls, partials


def _plan_attn(block_cls):
    """Per (g, kt): lead/trail skip blocks + strip tile range used."""
    plan = {}
    jmin, jmax = 31, 0
    for g in range(NG):
        kts = []
        for kt in range(NB):
            cls4 = [block_cls[kt][4 * g + j] for j in range(4)]
            if all(c == 0 for c in cls4):
                continue
            lead = 0
            while cls4[lead] == 0:
                lead += 1
            trail = 0
            while cls4[3 - trail] == 0:
                trail += 1
            jj0 = 4 * g - kt + 15
            jmin = min(jmin, jj0 + lead)
            jmax = max(jmax, jj0 + 3 - trail)
            kts.append((kt, lead, trail, cls4))
        plan[g] = kts
    if jmin > jmax:
        jmin, jmax = 0, 0
    return plan, jmin, jmax


def _get_compiled(key, mask, b1_zero, g2_trivial):
    if key in _CACHE:
        return _CACHE[key]
    block_cls, partials = _classify_mask(mask)
    plan, jmin, jmax = _plan_attn(block_cls)
    n_strip = jmax - jmin + 1
    nca = build_kernel_a(plan, jmin, n_strip, len(partials), b1_zero)
    nca.compile()
    ncb = build_kernel_b(g2_trivial)
    ncb.compile()
    _CACHE[key] = (nca, ncb, jmin, n_strip, partials)
    return _CACHE[key]


def _chunk_major(a, nk):
    """[nk*128, F] -> [128, nk, F] (partition-major chunk layout)."""
    f = a.shape[1]
    return np.ascontiguousarray(a.reshape(nk, P, f).transpose(1, 0, 2))


def kernel(x, mask, w1, b1, w2, b2, g1, beta1, g2, beta2, pos_w):
    x = np.asarray(x, np.float32)
    w1 = np.asarray(w1, np.float32)
    b1 = np.asarray(b1, np.float32)
    w2 = np.asarray(w2, np.float32)
    b2 = np.asarray(b2, np.float32)
    g1 = np.asarray(g1, np.float32)
    beta1 = np.asarray(beta1, np.float32)
    g2 = np.asarray(g2, np.float32)
    beta2 = np.asarray(beta2, np.float32)
    pos_w = np.asarray(pos_w, np.float32)
    mask_np = np.asarray(mask)

    b1_zero = bool(np.all(b1 == 0.0))
    g2_trivial = bool(np.all(g2 == 1.0) and np.all(beta2 == 0.0))
    key = (mask_np.tobytes(), b1_zero, g2_trivial)
    nca, ncb, jmin, n_strip, partials = _get_compiled(
        key, mask_np, b1_zero, g2_trivial)

    trace = bool(int(os.environ.get("HSTU_TRACE", "0")))
    strip = np.zeros((P, n_strip * P), np.float32)
    pidx = np.arange(P)[:, None]
    fidx = np.arange(P)[None, :]
    for i in range(n_strip):
        jj = jmin + i
        base = M - 1 - P * (jj - 15)
        strip[:, i * P:(i + 1) * P] = pos_w[base + pidx - fidx]
    ident = np.eye(P, dtype=nbf)
    inv128 = np.full((P, P), 1.0 / P, nbf)
    zeros_t = np.zeros((P, 512), nbf)
    parts_arr = (np.concatenate(partials, axis=1).astype(nbf) if partials
                 else np.zeros((P, P), nbf))

    xT = [np.ascontiguousarray(x[b].T) for b in range(B)]
    # [128, NG, 8, 512]: slab-major then k-chunk so one DMA covers a slab
    xTrs = [np.ascontiguousarray(
        t.astype(nbf).reshape(8, P, NG, 512).transpose(1, 2, 0, 3))
        for t in xT]
    in_maps_a = []
    for c in range(8):
        b, hg = divmod(c, 4)
        heads = [4 * hg + i for i in range(4)]
        rows_q = np.concatenate([np.arange(D + h * HD, D + (h + 1) * HD)
                                 for h in heads])
        rows_k = np.concatenate([np.arange(2 * D + h * HD, 2 * D + (h + 1) * HD)
                                 for h in heads])
        rows_u = np.concatenate([np.arange(h * HD, (h + 1) * HD) for h in heads])
        rows_v = np.concatenate([np.arange(3 * D + h * HD, 3 * D + (h + 1) * HD)
                                 for h in heads])
        rows_qku = np.concatenate([rows_q, rows_k, rows_u])
        b1_2d = np.ascontiguousarray(b1[rows_qku].reshape(6, P).T)
        in_maps_a.append(dict(
            xTr=xTrs[b],
            w1qr=np.ascontiguousarray(
                w1[rows_qku].T.astype(nbf).reshape(8, P, 6, P)
                .transpose(1, 2, 0, 3)),
            wvr=_chunk_major(
                np.ascontiguousarray(w1[rows_v].T).astype(nbf), 8),
            b1_2d=b1_2d,
            b1v_bc=np.ascontiguousarray(
                np.broadcast_to(b1[rows_v][None, :], (P, DC))).astype(nbf),
            inv128=inv128, ident=ident, strip=strip.astype(nbf),
            zeros_t=zeros_t, parts=parts_arr,
        ))
    res_a = run_bass_kernel_spmd(nca, in_maps_a, core_ids=list(range(8)),
                                 trace=trace)

    yT_full = [np.empty((D, S), nbf) for _ in range(B)]
    for c in range(8):
        b, hg = divmod(c, 4)
        yT_full[b][hg * DC:(hg + 1) * DC] = res_a.results[c]["yT_out"]

    # fold ln1's g1/beta1 and f2's b2 into the weights / residual
    w2g = w2 * g1[None, :]
    b2f = b2 + w2 @ beta1
    w2gT = np.ascontiguousarray(w2g.T).astype(nbf)
    wsr = np.ascontiguousarray(w2g.sum(axis=1)[None, :])
    gv2 = np.concatenate([g2.reshape(8, P).T, beta2.reshape(8, P).T], axis=1)
    gv2 = np.ascontiguousarray(gv2)
    ones_bm = np.ones((P, P), nbf)
    ones_rm = np.ones((P, P), np.float32)
    epsc = np.full((P, 1), EPS, np.float32)
    in_maps_b = []
    for c in range(8):
        b, qc = divmod(c, 4)
        sl = slice(qc * 512, (qc + 1) * 512)
        in_maps_b.append(dict(
            yr=_chunk_major(np.ascontiguousarray(yT_full[b][:, sl]), 8),
            w2r=_chunk_major(w2gT, 8),
            xbr=_chunk_major(
                np.ascontiguousarray(xT[b][:, sl] + b2f[:, None]), 8),
            wsr=wsr, gv2=gv2, ones_b=ones_bm, ones_r=ones_rm, epsc=epsc,
        ))
    res_b = run_bass_kernel_spmd(ncb, in_maps_b, core_ids=list(range(8)),
                                 trace=trace)

    out = np.empty((B, S, D), np.float32)
    for c in range(8):
        b, qc = divmod(c, 4)
        out[b, qc * 512:(qc + 1) * 512] = res_b.results[c]["outT"].T
    kernel.last_results = (res_a, res_b)
    return out
